# revision 2
# baseline (speedup 1.0000x reference)
"""Causal MHA (B=4, S=2048, D=1024, H=16) on 8 TRN2 cores, head-parallel,
fp8 DoubleRow edition.

Core c = (batch b=c//2, head-half hh=c%2). Same schedule skeleton as the
bf16 baseline, but every projection matmul runs fp8e4m3 DoubleRow:

- QKV/O projections: 3-term compensated digits (x = x8+xr, 16W = W8+Wr,
  dropping the xr*Wr term) -> bf16-level accuracy at 6/8 the bf16 PE cost.
  Digit pairs pack into DR slots: main instrs pair (W8_c, W8_c+1) x
  (x8_c, x8_c+1); cross instrs pair (Wr_c, W8_c) x (x8_c, xr_c).
- Scores: twin-sample split q16 = A+B (A = e4(q16/2), B = e4(q16-A)),
  k16 = K1+K2; one DR instr per head per k-tile computes K1.A + K2.B
  ~= q16.k16/2 with ~2.5% rms error (vs 3.6% single-digit), at HALF the
  bf16 score cost. exp scale absorbs the 2/(256*8) factor.
- ctx + transpose stay bf16 (e-quantization to fp8 would break the 2e-2
  gate). ctx values carry a 16x scale (v = x@(16Wv)); the oproj digits
  c8/cr quantize the 16x-scaled ctx (good e4m3 range), and the host
  divides partial outputs by 256.

fp8 digit production rides the idle GpSimd(Pool) engine + DVE slack.
"""

import os
import sys

sys.path.insert(0, "/opt/trn_rl_repo")

import numpy as np
import ml_dtypes

import concourse.bass as bass
import concourse.bacc as bacc
import concourse.tile as tile
from concourse import mybir
from concourse.bass_utils import run_bass_kernel_spmd

B, S, D, H = 4, 2048, 1024, 16
HD = D // H  # 64
P = 128
KC = D // P   # 8 contraction chunks for QKV projections
KC2 = 4       # contraction chunks for O projection (512 dims)
QW = 512      # query stripe width
NS = S // QW  # 4 stripes
NEG = -1e30
BF16 = mybir.dt.bfloat16
F32 = mybir.dt.float32
F8 = mybir.dt.float8e4
NPBF16 = ml_dtypes.bfloat16
NPE4 = ml_dtypes.float8_e4m3
WS = 16.0           # host weight scale
B_EARLY = int(os.environ.get("B_EARLY", "600"))   # stripe 0/1 interleave
B_MID = int(os.environ.get("B_MID", "700"))       # stripe 1 tail + stripe 2
B_S3 = int(os.environ.get("B_S3", "900"))         # stripe 3 pairs 0-2
B_LAST = int(os.environ.get("B_LAST", "1300"))    # final pair
B_CARRY = int(os.environ.get("B_CARRY", "1500"))  # carry cap
ESCALE = 2.0 / (WS * WS * 8.0)   # exp scale: twin-slot 2x / (16*16 * sqrt(hd))
DR = mybir.MatmulPerfMode.DoubleRow


def _build():
    nc = bacc.Bacc()

    # x digits, stripe-major: [indim-part, stripe, chunk, {x8,xr}, tok]
    xt = nc.declare_dram_parameter("xt", [P, NS, KC, 2, QW], F8, isOutput=False)
    # Wq/Wk digits, m-major: [indim-part, m, chunk, {Wr,W8}, 128 outdim]
    wq = nc.declare_dram_parameter("wq", [P, KC2, KC, 2, P], F8, isOutput=False)
    wk = nc.declare_dram_parameter("wk", [P, KC2, KC, 2, P], F8, isOutput=False)
    # Wv digits, chunk-major: [indim-part, chunk, {Wr,W8}, 512 outdim]
    wv = nc.declare_dram_parameter("wv", [P, KC, 2, 512], F8, isOutput=False)
    # Wo digits: [dh-part, chunk(=pair), {Wor,Wo8}, dout]
    wo = nc.declare_dram_parameter("wo", [P, KC2, 2, D], F8, isOutput=False)
    bqp = nc.declare_dram_parameter("bqp", [P, KC2], F32, isOutput=False)
    bkp = nc.declare_dram_parameter("bkp", [P, KC2], F32, isOutput=False)
    pad = nc.declare_dram_parameter("pad", [P, S // P], F32, isOutput=False)
    tri = nc.declare_dram_parameter("tri", [P, QW], BF16, isOutput=False)
    eye = nc.declare_dram_parameter("eye", [P, P], BF16, isOutput=False)
    out = nc.declare_dram_parameter("out", [S, D], BF16, isOutput=True)

    from contextlib import ExitStack

    with tile.TileContext(nc) as tc, ExitStack() as ctx:
        wpool = ctx.enter_context(tc.tile_pool(name="wpool", bufs=1))
        xpool = ctx.enter_context(tc.tile_pool(name="xpool", bufs=2))
        bigpool = ctx.enter_context(tc.tile_pool(name="bigpool", bufs=1))
        epool = ctx.enter_context(tc.tile_pool(name="epool", bufs=34))
        spool = ctx.enter_context(tc.tile_pool(name="spool", bufs=6))
        pp_acc = ctx.enter_context(tc.tile_pool(name="pp_acc", bufs=2, space="PSUM"))
        pp_sc = ctx.enter_context(tc.tile_pool(name="pp_sc", bufs=2, space="PSUM"))
        pp_ctx = ctx.enter_context(tc.tile_pool(name="pp_ctx", bufs=1, space="PSUM"))

        # ---- constants into SBUF ----
        wq_s = wpool.tile([P, KC2, KC, 2, P], F8, tag="wq")
        wk_s = wpool.tile([P, KC2, KC, 2, P], F8, tag="wk")
        wv_s = wpool.tile([P, KC, 2, 512], F8, tag="wv")
        wo_s = wpool.tile([P, KC2, 2, D], F8, tag="wo")
        bq_s = wpool.tile([P, KC2], F32, tag="bq")
        bk_s = wpool.tile([P, KC2], F32, tag="bk")
        pad_s = wpool.tile([P, S // P], F32, tag="pad")
        tri_s = wpool.tile([P, QW], BF16, tag="tri")
        eye_s = wpool.tile([P, P], BF16, tag="eye")
        # touch Exp once at t=0 so the ~1.3us ACT table load happens inside
        # the startup DMA shadow, not at the first real softmax
        warm_s = wpool.tile([P, 1], F32, tag="warm")
        nc.vector.memset(warm_s[:], 0.0)
        nc.scalar.activation(warm_s[:], warm_s[:],
                             mybir.ActivationFunctionType.Exp, scale=1.0)

        # ---- big persistent activations ----
        # q digits [pairdims, pair, {A,B}, q]; k digits [pairdims, pair, {K1,K2}, k]
        qd_s = bigpool.tile([P, KC2, 2, S], F8, tag="qd")
        kd_s = bigpool.tile([P, KC2, 2, S], F8, tag="kd")
        v_s = bigpool.tile([P, S // P, 8, HD + 1], BF16, tag="v")  # [k, ktile, h, hd|1]
        nc.vector.memset(v_s[:, :, :, HD:HD + 1], 1.0)
        # ctx digits [pairdims, pair, {c8,cr}, q]
        cd_s = bigpool.tile([P, KC2, 2, S], F8, tag="cd")

        def load_xt(st):
            xt_t = xpool.tile([P, KC, 2, QW], F8, tag="xt")
            nc.sync.dma_start(xt_t[:], xt[:, st, :, :, :])
            return xt_t

        def qkv_tiles(st, xt_t, parts="qkv", on_act=False, ms=None,
                      dig_dve=False):
            """Thunks projecting tokens [st*512, (st+1)*512): Q stripe st,
            K/V k-tiles 4*st..4*st+3, all fp8 DoubleRow 3-term. Each
            (matrix, m) splits into 3 PE emission units (~427ns each):
            main(4 DR), crossA(4 DR), crossB(4 DR)+psum->digit handoff."""
            ssl = slice(st * QW, (st + 1) * QW)
            thunks = []
            state = {}

            def qk_unit(w_s, b_s, dst, m, part, key):
                # part 0: main pairs; part 1: cross c=0..3; part 2: cross c=4..7
                # then qbf (DVE) + digit A (pool) + digit B (pool)
                if part == 0:
                    state[key] = pp_acc.tile([P, QW], F32, tag="acc",
                                             name="acc_ps")
                    ps = state[key]
                    for c in (0, 2, 4, 6):
                        nc.tensor.matmul(
                            ps[:], lhsT=w_s[:, m, c:c + 2, 1, :],
                            rhs=xt_t[:, c:c + 2, 0, :],
                            start=(c == 0), stop=False, perf_mode=DR)
                else:
                    ps = state[key]
                    for c in range(4 * (part - 1), 4 * part):
                        nc.tensor.matmul(
                            ps[:], lhsT=w_s[:, m, c, 0:2, :],
                            rhs=xt_t[:, c, 0:2, :],
                            start=False, stop=(c == KC - 1), perf_mode=DR)
                    if part == 2:
                        qbf = spool.tile([P, QW], BF16, tag="qbf",
                                         name="qbf", bufs=3)
                        nc.vector.tensor_scalar_add(qbf[:], ps[:],
                                                    b_s[:, m:m + 1])
                        del state[key]
                        # DVE for the startup digits (Pool's Q7 launches
                        # would gate the first scores), Pool afterwards
                        eng = nc.vector if dig_dve else nc.gpsimd
                        eng.tensor_scalar_mul(
                            dst[:, m, 0, ssl], qbf[:], 0.5)
                        eng.tensor_tensor(
                            dst[:, m, 1, ssl], qbf[:], dst[:, m, 0, ssl],
                            mybir.AluOpType.subtract)

            def v_unit(sub, part, key):
                subsl = slice(sub * P, (sub + 1) * P)
                if part == 0:
                    state[key] = pp_acc.tile([P, 8, HD], F32, tag="acc",
                                             name="acc_ps")
                    ps = state[key]
                    for c in (0, 2, 4, 6):
                        nc.tensor.matmul(
                            ps[:], lhsT=xt_t[:, c:c + 2, 0, subsl],
                            rhs=wv_s[:, c:c + 2, 1, :],
                            start=(c == 0), stop=False, perf_mode=DR)
                else:
                    ps = state[key]
                    for c in range(4 * (part - 1), 4 * part):
                        nc.tensor.matmul(
                            ps[:], lhsT=xt_t[:, c, 0:2, subsl],
                            rhs=wv_s[:, c, 0:2, :],
                            start=False, stop=(c == KC - 1), perf_mode=DR)
                    if part == 2:
                        nc.vector.tensor_copy(
                            out=v_s[:, st * 4 + sub, :, 0:HD], in_=ps[:])
                        del state[key]

            plan = []
            if "q" in parts:
                plan.append(("q", wq_s, bq_s, qd_s))
            if "k" in parts:
                plan.append(("k", wk_s, bk_s, kd_s))
            for pn, w_s, b_s, dst in plan:
                for m in (range(KC2) if ms is None else ms):
                    for part in range(3):
                        tag = (st, pn, m) if part == 2 else None
                        thunks.append(
                            (427, lambda w_s=w_s, b_s=b_s, dst=dst, m=m,
                             part=part, key=(pn, m):
                             qk_unit(w_s, b_s, dst, m, part, key), tag))
            if "v" in parts:
                for sub in range(4):
                    for part in range(3):
                        tag = (st, "v", sub) if part == 2 else None
                        thunks.append((427, lambda sub=sub, part=part,
                                       key=("v", sub): v_unit(sub, part, key),
                                       tag))
            return thunks

        def qkv_stage(st, xt_t, parts="qkv", ms=None, dig_dve=False):
            for _, t, tag in qkv_tiles(st, xt_t, parts, ms=ms,
                                       dig_dve=dig_dve):
                t()
                if tag:
                    done_tags.add(tag)

        from collections import deque
        fillers = deque()   # (ns, thunk, tag-or-None)
        done_tags = set()

        def pop_filler():
            ns, t, tag = fillers.popleft()
            t()
            if tag:
                done_tags.add(tag)
            return ns

        def ensure(tag):
            while tag not in done_tags and fillers:
                pop_filler()

        def drain():
            while fillers:
                pop_filler()

        attn_state = {}

        def attn_ph1(stripe, pr, j):
            """Scores/exp/mask for one (pair, k-tile); e kept in SBUF.
            One DR matmul per head: slots (K1,A)+(K2,B) ~= q16.k16/2."""
            es = attn_state[(stripe, pr)]
            m = j
            ksl = slice(m * P, (m + 1) * P)
            diag = m >= 4 * stripe
            # within a diagonal k-tile of shift t = m-4s, queries below
            # t*128 are entirely masked -- compute only the valid sub-range
            off = (m - 4 * stripe) * P if diag else 0
            w = QW - off
            qsub = slice(stripe * QW + off, (stripe + 1) * QW)
            sc = pp_sc.tile([P, 2 * QW], F32, tag="sc")
            for q_i in range(2):
                lo = q_i * HD
                nc.tensor.matmul(
                    sc[:, q_i * QW + off:(q_i + 1) * QW],
                    lhsT=kd_s[lo:lo + HD, pr, 0:2, ksl],
                    rhs=qd_s[lo:lo + HD, pr, 0:2, qsub],
                    start=True, stop=True, perf_mode=DR,
                    tile_position=(lo, 0))
            e = epool.tile([P, 2 * QW], BF16, tag="e")
            es.append(e)
            if off >= 256:
                # narrow diagonal exps: two short instructions beat one
                # full-width one once off >= 256 (ACT is the critical engine)
                for q_i in range(2):
                    esl = slice(q_i * QW + off, (q_i + 1) * QW)
                    nc.scalar.activation(e[:, esl], sc[:, esl],
                                         mybir.ActivationFunctionType.Exp,
                                         bias=pad_s[:, m:m + 1],
                                         scale=ESCALE)
            else:
                # full-width exp: any masked query columns hold garbage
                # (stale PSUM) but are never read downstream
                nc.scalar.activation(e[:], sc[:],
                                     mybir.ActivationFunctionType.Exp,
                                     bias=pad_s[:, m:m + 1],
                                     scale=ESCALE)
            if diag:
                # only the 128-wide diagonal query block needs the triangle;
                # beyond it tri is all-ones (no-op)
                for q_i in range(2):
                    esl = slice(q_i * QW + off, q_i * QW + off + P)
                    nc.vector.tensor_tensor(
                        e[:, esl], e[:, esl], tri_s[:, 0:P],
                        mybir.AluOpType.mult)

        lastcbf = {}

        def attn_ph2_parts(stripe, pr, last=False):
            """Post-phase-1 work for a pair. Returns {"region": fn,
            "units": [...]}. Normal pairs: units include the 8 ctx region
            bursts and end with the fp8 digit split of the transposed ctx.
            last=True: regions are called eagerly by the caller inside the
            ph1 j-loop, norms read PSUM directly (skip craw), and the digit
            split is skipped -- the tail oproj closes consume the bf16
            transpose staging buffer instead."""
            qsl = slice(stripe * QW, (stripe + 1) * QW)
            es = attn_state[(stripe, pr)]
            nkt = 4 * stripe + 4
            st = {}

            def region(q_i, qb):
                if (q_i, qb) == (0, 0):
                    st["ctx_ps"] = pp_ctx.tile([P, 2, QW], F32, tag="ctx",
                                               name="ctx_ps")
                ctx_ps = st["ctx_ps"]
                h = 2 * pr + q_i
                js = list(range(0, 4 * stripe + qb + 1))
                for i, j in enumerate(js):
                    nc.tensor.matmul(
                        ctx_ps[:, q_i, qb * (HD + 1):(qb + 1) * (HD + 1)],
                        lhsT=es[j][:, q_i * QW + qb * P:q_i * QW + (qb + 1) * P],
                        rhs=v_s[:, j, h, :],
                        start=(i == 0), stop=(i == len(js) - 1),
                        skip_group_check=True)

            def recip():
                st["rden"] = spool.tile([P, 2, 4], F32, tag="rden",
                                        name="rden", bufs=2)
                nc.vector.reciprocal(
                    st["rden"][:],
                    st["ctx_ps"][:, :, HD:4 * (HD + 1):HD + 1])

            def recip_qb(qb):
                if "rden" not in st:
                    st["rden"] = spool.tile([P, 2, 4], F32, tag="rden",
                                            name="rden", bufs=2)
                nc.vector.reciprocal(
                    st["rden"][:, :, qb:qb + 1],
                    st["ctx_ps"][:, :, qb * (HD + 1) + HD:
                                 qb * (HD + 1) + HD + 1])

            def copyraw():
                # one bf16 copy frees the 2-bank ctx slot immediately; the
                # per-region normalizes then read SBUF at 2x DVE rate
                st["craw"] = spool.tile([P, 2, QW], BF16, tag="craw",
                                        name="craw", bufs=2)
                nc.vector.tensor_copy(out=st["craw"][:], in_=st["ctx_ps"][:])
                del st["ctx_ps"]
                attn_state[(stripe, pr)] = []  # release e tiles

            def norm(q_i, qb):
                if last:
                    src = st["ctx_ps"][:, q_i, qb * (HD + 1):qb * (HD + 1) + HD]
                else:
                    src = st["craw"][:, q_i, qb * (HD + 1):qb * (HD + 1) + HD]
                nc.vector.tensor_scalar_mul(
                    st["ctxc"][:, (q_i * 4 + qb) * HD:(q_i * 4 + qb + 1) * HD],
                    src,
                    st["rden"][:, q_i, qb:qb + 1])

            def mkctxc():
                st["ctxc"] = spool.tile([P, QW], BF16, tag="ctxc",
                                        name="ctxc", bufs=2)

            def transpose():
                st["pt"] = pp_ctx.tile([P, 2, QW], F32, tag="ctx", name="pt")
                for q_i in range(2):
                    for qb in range(4):
                        nc.tensor.matmul(
                            st["pt"][q_i * HD:(q_i + 1) * HD, 0,
                                     qb * P:(qb + 1) * P],
                            lhsT=st["ctxc"][:, (q_i * 4 + qb) * HD:
                                            (q_i * 4 + qb + 1) * HD],
                            rhs=eye_s[:],
                            start=True, stop=True,
                            skip_group_check=True)

            def ptcopy():
                # stage psum -> bf16, then split to fp8 digits on Pool
                st["cbf"] = spool.tile([P, QW], BF16, tag="cbf",
                                       name="cbf", bufs=2)
                nc.vector.tensor_copy(out=st["cbf"][:], in_=st["pt"][:, 0, :])
                del st["pt"]
                if last:
                    lastcbf[0] = st["cbf"]
                    del attn_state[(stripe, pr)]

            def cdig():
                nc.gpsimd.tensor_scalar_mul(cd_s[:, pr, 0, qsl],
                                            st["cbf"][:], 1.0)
                nc.gpsimd.tensor_tensor(cd_s[:, pr, 1, qsl], st["cbf"][:],
                                        cd_s[:, pr, 0, qsl],
                                        mybir.AluOpType.subtract)
                del attn_state[(stripe, pr)]

            if last:
                # per-qb finish: reciprocal + both heads' normalizes as soon
                # as that qb's denominator lands, inside the exp shadow
                def early_finish(qb):
                    if qb == 0:
                        mkctxc()
                    recip_qb(qb)
                    norm(0, qb)
                    norm(1, qb)

                units = [(0, lambda: st.pop("ctx_ps")),
                         (427, transpose), (0, ptcopy)]
                return {"region": region, "early_finish": early_finish,
                        "units": units}
            units = [(27 * (4 * stripe + qb + 1),
                      lambda q_i=q_i, qb=qb: region(q_i, qb))
                     for q_i in range(2) for qb in range(4)]
            units += [(0, recip), (0, copyraw), (0, mkctxc)]
            units += [(0, lambda q_i=q_i, qb=qb: norm(q_i, qb))
                      for q_i in range(2) for qb in range(4)]
            units += [(427, transpose), (0, ptcopy), (0, cdig)]
            return {"region": region, "units": units}

        def attn_ph2_units(stripe, pr):
            return attn_ph2_parts(stripe, pr)["units"]

        pending = deque()  # ph2 units of the previously finished pair
        carry = [0]        # un-met PE deficit banked across k-tiles

        def attn_pair(stripe, pr, budget_ns=600, last=False):
            """Phase 1 j-loop for one pair, interleaving ph2 units of the
            previous pair (and fillers) between k-tiles. The exp costs
            ~1038ns/k-tile vs ~213ns of DR scores, so ~800ns of PE filler
            per k-tile keeps the PE dense; unmet remainder banks forward.
            last=True runs this pair's ctx regions eagerly inside the j-loop
            and its normalize/transpose chain immediately after, shortening
            the post-exp tail."""
            nkt = 4 * stripe + 4
            attn_state[(stripe, pr)] = []
            parts = attn_ph2_parts(stripe, pr, last=True) if last else None
            for j in range(nkt):
                attn_ph1(stripe, pr, j)
                if last and j >= 4 * stripe:
                    qb = j - 4 * stripe
                    if qb == 0:
                        # the previous pair's ph2 must be fully emitted
                        # before this pair claims the single ctx PSUM slot
                        while pending:
                            pending.popleft()[1]()
                    parts["region"](0, qb)
                    parts["region"](1, qb)
                    parts["early_finish"](qb)
                budget = carry[0] + budget_ns
                while budget > 100:
                    if pending:
                        ns, t = pending.popleft()
                        t()
                    elif fillers:
                        ns = pop_filler()
                    else:
                        break
                    budget -= max(ns, 50)
                carry[0] = min(max(budget, 0), B_CARRY)
            if last:
                for _, t in parts["units"]:
                    t()
            else:
                pending.extend(attn_ph2_units(stripe, pr))

        def oproj_tiles(t8, alt=False, quarters=False):
            """Thunks for one 128-token output block: per 512-dout half,
            6 DR instrs (2 main chunk-pairs + 4 cross), ~640ns, or three
            ~213ns thirds (quarters=True)."""
            osl = slice(t8 * P, (t8 + 1) * P)
            state = {}

            def mm_main(ps, cs, start):
                for c in cs:
                    nc.tensor.matmul(ps[:], lhsT=cd_s[:, c:c + 2, 0, osl],
                                     rhs=wo_s[:, c:c + 2, 1,
                                              state["dsl"]],
                                     start=(start and c == cs[0]), stop=False,
                                     perf_mode=DR)

            def mm_cross(ps, kcs, stop):
                for kc in kcs:
                    nc.tensor.matmul(ps[:], lhsT=cd_s[:, kc, 0:2, osl],
                                     rhs=wo_s[:, kc, 0:2, state["dsl"]],
                                     start=False, stop=(stop and kc == kcs[-1]),
                                     perf_mode=DR)

            def fin(dt, ps):
                # both 512-halves stage into one tile; a single [128, 1024]
                # DMA per token block halves HWDGE/semaphore traffic
                if dt == 0:
                    state["ob"] = spool.tile([P, 2, QW], BF16, tag="outsb",
                                             name="ob", bufs=3)
                ob = state["ob"]
                nc.vector.tensor_copy(out=ob[:, dt, :], in_=ps[:])
                if dt == 1:
                    (nc.scalar if alt else nc.sync).dma_start(
                        out[osl, :], ob[:])
                    del state["ob"]

            def whole(dt):
                state["dsl"] = slice(dt * QW, (dt + 1) * QW)
                if alt and dt == 1:
                    ps = pp_sc.tile([P, 2 * QW], F32, tag="sc",
                                    name="oproj_ps")[:, 0:QW]
                else:
                    ps = pp_acc.tile([P, QW], F32, tag="acc", name="oproj_ps")
                mm_main(ps, [0, 2], True)
                mm_cross(ps, range(KC2), True)
                fin(dt, ps)

            def qopen(dt):
                state["dsl"] = slice(dt * QW, (dt + 1) * QW)
                state[dt] = pp_acc.tile([P, QW], F32, tag="acc",
                                        name="oproj_ps")
                mm_main(state[dt], [0, 2], True)
                mm_cross(state[dt], [0, 1], False)

            def qclose(dt):
                state["dsl"] = slice(dt * QW, (dt + 1) * QW)
                ps = state.pop(dt)
                mm_cross(ps, [2, 3], True)
                fin(dt, ps)

            if quarters:
                return [lambda dt=dt, f=f: f(dt)
                        for dt in range(2) for f in (qopen, qclose)]
            return [lambda dt=dt: whole(dt) for dt in range(2)]

        def oproj(t8, alt=False):
            for t in oproj_tiles(t8, alt):
                t()

        heldpart = {}

        def oproj_openA(t8, dt):
            """Pairs 0/1 products of a final oproj tile -- legal as soon as
            cdig(3,1) has popped, i.e. inside pair (3,2)'s window where the
            PE otherwise starves. Staged to SBUF bf16; the close replays
            pair 2 + the bf16 last-pair products and adds this partial."""
            ps = pp_acc.tile([P, QW], F32, tag="acc", name="oproj_ps")
            dsl = slice(dt * QW, (dt + 1) * QW)
            osl = slice(t8 * P, (t8 + 1) * P)
            nc.tensor.matmul(ps[:], lhsT=cd_s[:, 0:2, 0, osl],
                             rhs=wo_s[:, 0:2, 1, dsl],
                             start=True, stop=False, perf_mode=DR)
            for kc in range(2):
                nc.tensor.matmul(ps[:], lhsT=cd_s[:, kc, 0:2, osl],
                                 rhs=wo_s[:, kc, 0:2, dsl],
                                 start=False, stop=(kc == 1), perf_mode=DR)
            part = spool.tile([P, QW], BF16, tag="opart",
                              name=f"opart{t8}_{dt}", bufs=8)
            nc.vector.tensor_copy(out=part[:], in_=ps[:])
            heldpart[(t8, dt)] = part

        heldob = {}

        heldps = {}

        def oproj_close(t8, dt):
            part = heldpart.pop((t8, dt))
            cbf = lastcbf[0]
            tloc = slice((t8 - 12) * P, (t8 - 11) * P)
            ps = pp_acc.tile([P, QW], F32, tag="acc", name="oproj_ps")
            dsl = slice(dt * QW, (dt + 1) * QW)
            osl = slice(t8 * P, (t8 + 1) * P)
            nc.tensor.matmul(ps[:], lhsT=cd_s[:, 2, 0:2, osl],
                             rhs=wo_s[:, 2, 0:2, dsl],
                             start=True, stop=False, perf_mode=DR)
            nc.tensor.matmul(ps[:], lhsT=cd_s[:, 2, 0, osl],
                             rhs=wo_s[:, 2, 1, dsl],
                             start=False, stop=False)
            nc.tensor.matmul(ps[:], lhsT=cbf[:, tloc],
                             rhs=wo_s[:, 3, 1, dsl],
                             start=False, stop=False)
            nc.tensor.matmul(ps[:], lhsT=cbf[:, tloc],
                             rhs=wo_s[:, 3, 0, dsl],
                             start=False, stop=True)
            if dt == 0:
                heldob[t8] = spool.tile([P, 2, QW], BF16, tag="outsb",
                                        name="ob", bufs=3)
            ob = heldob[t8]
            nc.vector.tensor_tensor(ob[:, dt, :], ps[:], part[:],
                                    mybir.AluOpType.add)
            # per-half DMA: each half ships as soon as its add lands
            (nc.sync if t8 % 2 else nc.scalar).dma_start(out[osl, dsl],
                                                         ob[:, dt, :])
            if dt == 1:
                del heldob[t8]

        # ---- schedule ----
        # Startup DMAs: the minimal set for pair (0,0) first (xt0, m0 of
        # Wq/Wk, biases, pad, tri), then attention starts while the rest of
        # the weights stream in and the remaining stage work rides fillers.
        xt0 = xpool.tile([P, KC, 2, QW], F8, tag="xt")
        nc.scalar.dma_start(xt0[:, 0:2, :, :], xt[:, 0, 0:2, :, :])
        nc.sync.dma_start(xt0[:, 4:6, :, :], xt[:, 0, 4:6, :, :])
        # PE warmup on a zeroed tile: keeps the PE continuously busy through
        # the startup DMA shadow so the first real matmuls run at full clock
        # (the cost model's p-state ramp needs ~3us of uninterrupted work)
        wmm = wpool.tile([P, QW], BF16, tag="wmm")
        nc.gpsimd.memset(wmm[:], 0.0)
        wps = pp_sc.tile([P, 2 * QW], F32, tag="sc", name="warm_ps")
        for i in range(8):
            nc.tensor.matmul(wps[:, 0:QW], lhsT=wmm[:, 0:P], rhs=wmm[:],
                             start=(i == 0), stop=(i == 7))
        nc.scalar.dma_start(xt0[:, 2:4, :, :], xt[:, 0, 2:4, :, :])
        nc.sync.dma_start(wk_s[:, 0, :, :, :], wk[:, 0, :, :, :])
        nc.scalar.dma_start(xt0[:, 6:8, :, :], xt[:, 0, 6:8, :, :])
        nc.sync.dma_start(wq_s[:, 0, :, :, :], wq[:, 0, :, :, :])
        nc.scalar.dma_start(bk_s[:], bkp[:])
        nc.sync.dma_start(bq_s[:], bqp[:])
        nc.scalar.dma_start(tri_s[:], tri[:])
        nc.sync.dma_start(pad_s[:], pad[:])
        # q digits on DVE, k digits on Pool: the two first-pair digit chains
        # run on different engines in parallel
        qkv_stage(0, xt0, parts="q", ms=[0], dig_dve=True)
        qkv_stage(0, xt0, parts="k", ms=[0])
        nc.sync.dma_start(wv_s[:, 0:4, :, :], wv[:, 0:4, :, :])
        nc.scalar.dma_start(wv_s[:, 4:8, :, :], wv[:, 4:8, :, :])
        xt1 = load_xt(1)
        nc.sync.dma_start(wq_s[:, 1, :, :, :], wq[:, 1, :, :, :])
        nc.scalar.dma_start(wk_s[:, 1, :, :, :], wk[:, 1, :, :, :])
        nc.sync.dma_start(wq_s[:, 2, :, :, :], wq[:, 2, :, :, :])
        nc.scalar.dma_start(wk_s[:, 2, :, :, :], wk[:, 2, :, :, :])
        nc.sync.dma_start(wq_s[:, 3, :, :, :], wq[:, 3, :, :, :])
        nc.scalar.dma_start(wk_s[:, 3, :, :, :], wk[:, 3, :, :, :])
        nc.scalar.dma_start(eye_s[:], eye[:])

        # stripe 0/1 pair interleave: stripe 0 alone supplies too little exp
        # work to keep ACT busy through the projection-heavy opening, so
        # stripe-1 pairs (2x the exp volume) run in between
        fillers.extend(qkv_tiles(1, xt1, parts="q", ms=[0]))
        fillers.extend(qkv_tiles(1, xt1, parts="k", ms=[0]))
        fillers.extend(qkv_tiles(0, xt0, parts="v"))
        fillers.extend(qkv_tiles(0, xt0, parts="q", ms=[1]))
        fillers.extend(qkv_tiles(0, xt0, parts="k", ms=[1]))
        fillers.extend(qkv_tiles(1, xt1, parts="v"))
        for m in (1, 2, 3):
            sts = (1,) if m == 1 else (0, 1)
            for stq in sts:
                xtt = xt0 if stq == 0 else xt1
                fillers.extend(qkv_tiles(stq, xtt, parts="q", ms=[m]))
                fillers.extend(qkv_tiles(stq, xtt, parts="k", ms=[m]))
        nc.sync.dma_start(wo_s[:], wo[:])
        attn_pair(0, 0, budget_ns=900)
        ensure((1, "q", 0))
        ensure((1, "k", 0))
        ensure((0, "v", 3))
        attn_pair(1, 0, budget_ns=B_EARLY)
        ensure((0, "q", 1))
        ensure((0, "k", 1))
        ensure((1, "v", 3))
        attn_pair(0, 1, budget_ns=B_EARLY)
        ensure((1, "q", 1))
        ensure((1, "k", 1))
        attn_pair(1, 1, budget_ns=B_EARLY)
        ensure((0, "q", 2))
        ensure((0, "k", 2))
        attn_pair(0, 2, budget_ns=B_EARLY)
        ensure((1, "q", 2))
        ensure((1, "k", 2))
        attn_pair(1, 2, budget_ns=B_EARLY)
        ensure((0, "q", 3))
        ensure((0, "k", 3))
        attn_pair(0, 3, budget_ns=B_EARLY)
        ensure((1, "q", 3))
        ensure((1, "k", 3))
        xt2 = load_xt(2)

        def extend_stage_qk(st, xt_t):
            for m in range(KC2):
                fillers.extend(qkv_tiles(st, xt_t, parts="q", ms=[m]))
                fillers.extend(qkv_tiles(st, xt_t, parts="k", ms=[m]))

        extend_stage_qk(2, xt2)
        fillers.extend(qkv_tiles(2, xt2, parts="v"))
        attn_pair(1, 3, budget_ns=B_EARLY)
        xt3 = load_xt(3)
        extend_stage_qk(3, xt3)
        for pr in range(4):
            ensure((2, "q", pr))
            ensure((2, "k", pr))
            if pr == 1:
                ensure((2, "v", 3))
            attn_pair(2, pr, budget_ns=B_MID)
        # V(3) is safe here -- first needed by the ctx regions of pair (3,0),
        # which only run during ph1(3,1). oproj of stripe-2 blocks becomes
        # legal once pair (2,3)'s pending units pop at the start of (3,0).
        fillers.extend(qkv_tiles(3, xt3, parts="v"))
        for t8 in range(0, 11):
            fillers.extend((213, t, None)
                           for t in oproj_tiles(t8, quarters=True))
        for pr in range(2):
            ensure((3, "q", pr))
            ensure((3, "k", pr))
            if pr == 1:
                ensure((3, "v", 3))
            attn_pair(3, pr, budget_ns=B_S3)
        # openA (pairs 0/1 products) of the final oproj tiles becomes legal
        # once cdig(3,1) pops -- feed it to pair (3,2)'s otherwise-starved
        # PE, together with the remaining stripe-2 oproj tile
        ensure((3, "q", 2))
        ensure((3, "k", 2))
        fillers.extend((213, t, None)
                       for t in oproj_tiles(11, quarters=True))
        for t8 in range(12, 16):
            fillers.append((320, (lambda t8=t8: oproj_openA(t8, 0)), None))
            fillers.append((320, (lambda t8=t8: oproj_openA(t8, 1)), None))
        attn_pair(3, 2, budget_ns=B_S3)
        ensure((3, "q", 3))
        ensure((3, "k", 3))
        attn_pair(3, 3, budget_ns=B_LAST, last=True)
        while pending:
            pending.popleft()[1]()
        drain()
        for t8 in range(12, 16):
            oproj_close(t8, 0)
            oproj_close(t8, 1)

    nc.compile()
    return nc


def _dig(a):
    """two-digit e4m3 split along a new axis: returns np [..., 2] fp8"""
    hi = a.astype(NPE4)
    lo = (a - hi.astype(np.float32)).astype(NPE4)
    return hi, lo


def _core_inputs(c, x, padding_mask, Wq, bq, Wk, bk, Wv, bv, Wo, bo):
    b, hh = c // 2, c % 2
    hsl = slice(hh * 512, (hh + 1) * 512)

    xb = np.ascontiguousarray(
        x[b].T.reshape(KC, P, S).transpose(1, 0, 2)).astype(np.float32)
    x8, xr = _dig(xb)
    # [P, KC, 2, S] -> stripe-major [P, NS, KC, 2, QW]
    xt = np.stack([x8, xr], axis=2).reshape(P, KC, 2, NS, QW)
    xt = np.ascontiguousarray(xt.transpose(0, 3, 1, 2, 4))

    def wl(Wh):  # [512 out, 1024 in] -> m-major [P, KC2, KC, 2, 128] {Wr,W8}
        w = np.ascontiguousarray(
            Wh.T.reshape(KC, P, 512).transpose(1, 0, 2)).astype(np.float32)
        w8, wr = _dig(WS * w)
        st = np.stack([wr, w8], axis=2)          # [P, KC, 2, 512]
        st = st.reshape(P, KC, 2, KC2, P).transpose(0, 3, 1, 2, 4)
        return np.ascontiguousarray(st)

    def wvl(Wh):  # [512 out, 1024 in] -> chunk-major [P, KC, 2, 512] {Wr,W8}
        w = np.ascontiguousarray(
            Wh.T.reshape(KC, P, 512).transpose(1, 0, 2)).astype(np.float32)
        w8, wr = _dig(WS * w)
        return np.ascontiguousarray(np.stack([wr, w8], axis=2))

    wob = np.ascontiguousarray(
        Wo[:, hsl].T.reshape(KC2, P, D).transpose(1, 0, 2)).astype(np.float32)
    wo8, wor = _dig(WS * wob)
    wol = np.ascontiguousarray(np.stack([wor, wo8], axis=2))

    bqp = np.ascontiguousarray(
        WS * bq[hsl].reshape(KC2, P).T).astype(np.float32)
    bkp = np.ascontiguousarray(
        WS * bk[hsl].reshape(KC2, P).T).astype(np.float32)

    padb = np.where(padding_mask[b].reshape(S // P, P).T, 0.0,
                    NEG).astype(np.float32)
    padb = np.ascontiguousarray(padb)

    kk = np.arange(P)[:, None]
    uu = np.arange(QW)[None, :]
    trib = np.ascontiguousarray((kk <= uu).astype(NPBF16))

    return {"xt": xt, "wq": wl(Wq[hsl]), "wk": wl(Wk[hsl]), "wv": wvl(Wv[hsl]),
            "wo": wol, "bqp": bqp, "bkp": bkp, "pad": padb, "tri": trib,
            "eye": np.eye(P, dtype=NPBF16)}


_NC_CACHE = {}


def kernel(x, padding_mask, Wq, bq, Wk, bk, Wv, bv, Wo, bo):
    x = np.asarray(x, np.float32)
    padding_mask = np.asarray(padding_mask, bool)
    args = [np.asarray(a, np.float32) for a in (Wq, bq, Wk, bk, Wv, bv, Wo, bo)]

    if "nc" not in _NC_CACHE:
        _NC_CACHE["nc"] = _build()
    nc = _NC_CACHE["nc"]

    in_maps = [_core_inputs(c, x, padding_mask, *args) for c in range(8)]

    trace = bool(int(os.environ.get("KERNEL_TRACE", "0")))
    try:
        res = run_bass_kernel_spmd(nc, in_maps, core_ids=list(range(8)), trace=trace)
    except ModuleNotFoundError:
        res = run_bass_kernel_spmd(nc, in_maps, core_ids=list(range(8)))
    if trace and res.exec_time_ns is not None:
        print(f"HW exec time: {res.exec_time_ns} ns")
        _NC_CACHE["exec_time_ns"] = res.exec_time_ns

    Wo_, bv_, bo_ = args[6], args[5], args[7]
    btot = (bo_ + Wo_ @ bv_).astype(np.float32)
    descale = 1.0 / (WS * WS)
    full = np.empty((B, S, D), np.float32)
    for b in range(B):
        full[b] = ((res.results[2 * b]["out"].astype(np.float32)
                    + res.results[2 * b + 1]["out"].astype(np.float32))
                   * descale + btot)
    return full


if __name__ == "__main__":
    rng = np.random.default_rng(0)
    x = rng.standard_normal((B, S, D), dtype=np.float32)
    lengths = rng.integers(S // 2, S + 1, size=(B,))
    pm = np.arange(S)[None, :] < lengths[:, None]
    std = 0.02
    ws = {n: (rng.standard_normal((D, D), dtype=np.float32) * std)
          for n in ("Wq", "Wk", "Wv", "Wo")}
    z = np.zeros((D,), np.float32)
    out = kernel(x, pm, ws["Wq"], z, ws["Wk"], z, ws["Wv"], z, ws["Wo"], z)
    print(out.shape, out.dtype, np.abs(out).mean())


# revision 3
# speedup vs baseline: 1.0002x; 1.0002x over previous
"""Causal MHA (B=4, S=2048, D=1024, H=16) on 8 TRN2 cores, head-parallel,
fp8 DoubleRow edition.

Core c = (batch b=c//2, head-half hh=c%2). Same schedule skeleton as the
bf16 baseline, but every projection matmul runs fp8e4m3 DoubleRow:

- QKV/O projections: 3-term compensated digits (x = x8+xr, 16W = W8+Wr,
  dropping the xr*Wr term) -> bf16-level accuracy at 6/8 the bf16 PE cost.
  Digit pairs pack into DR slots: main instrs pair (W8_c, W8_c+1) x
  (x8_c, x8_c+1); cross instrs pair (Wr_c, W8_c) x (x8_c, xr_c).
- Scores: twin-sample split q16 = A+B (A = e4(q16/2), B = e4(q16-A)),
  k16 = K1+K2; one DR instr per head per k-tile computes K1.A + K2.B
  ~= q16.k16/2 with ~2.5% rms error (vs 3.6% single-digit), at HALF the
  bf16 score cost. exp scale absorbs the 2/(256*8) factor.
- ctx + transpose stay bf16 (e-quantization to fp8 would break the 2e-2
  gate). ctx values carry a 16x scale (v = x@(16Wv)); the oproj digits
  c8/cr quantize the 16x-scaled ctx (good e4m3 range), and the host
  divides partial outputs by 256.

fp8 digit production rides the idle GpSimd(Pool) engine + DVE slack.
"""

import os
import sys

sys.path.insert(0, "/opt/trn_rl_repo")

import numpy as np
import ml_dtypes

import concourse.bass as bass
import concourse.bacc as bacc
import concourse.tile as tile
from concourse import mybir
from concourse.bass_utils import run_bass_kernel_spmd

B, S, D, H = 4, 2048, 1024, 16
HD = D // H  # 64
P = 128
KC = D // P   # 8 contraction chunks for QKV projections
KC2 = 4       # contraction chunks for O projection (512 dims)
QW = 512      # query stripe width
NS = S // QW  # 4 stripes
NEG = -1e30
BF16 = mybir.dt.bfloat16
F32 = mybir.dt.float32
F8 = mybir.dt.float8e4
NPBF16 = ml_dtypes.bfloat16
NPE4 = ml_dtypes.float8_e4m3
WS = 16.0           # host weight scale
B_EARLY = 600   # filler budget/k-tile: stripe 0/1 interleave
B_MID = 700     # stripe 1 tail + stripe 2
B_S3 = 900      # stripe 3 pairs 0-2
B_LAST = 1300   # final pair
B_CARRY = 1500  # carry cap
ESCALE = 2.0 / (WS * WS * 8.0)   # exp scale: twin-slot 2x / (16*16 * sqrt(hd))
DR = mybir.MatmulPerfMode.DoubleRow


def _build():
    nc = bacc.Bacc()

    # x digits, stripe-major: [indim-part, stripe, chunk, {x8,xr}, tok]
    xt = nc.declare_dram_parameter("xt", [P, NS, KC, 2, QW], F8, isOutput=False)
    # Wq/Wk digits, m-major: [indim-part, m, chunk, {Wr,W8}, 128 outdim]
    wq = nc.declare_dram_parameter("wq", [P, KC2, KC, 2, P], F8, isOutput=False)
    wk = nc.declare_dram_parameter("wk", [P, KC2, KC, 2, P], F8, isOutput=False)
    # Wv digits, chunk-major: [indim-part, chunk, {Wr,W8}, 512 outdim]
    wv = nc.declare_dram_parameter("wv", [P, KC, 2, 512], F8, isOutput=False)
    # Wo digits: [dh-part, chunk(=pair), {Wor,Wo8}, dout]
    wo = nc.declare_dram_parameter("wo", [P, KC2, 2, D], F8, isOutput=False)
    bqp = nc.declare_dram_parameter("bqp", [P, KC2], F32, isOutput=False)
    bkp = nc.declare_dram_parameter("bkp", [P, KC2], F32, isOutput=False)
    pad = nc.declare_dram_parameter("pad", [P, S // P], F32, isOutput=False)
    tri = nc.declare_dram_parameter("tri", [P, QW], BF16, isOutput=False)
    eye = nc.declare_dram_parameter("eye", [P, P], BF16, isOutput=False)
    out = nc.declare_dram_parameter("out", [S, D], BF16, isOutput=True)

    from contextlib import ExitStack

    with tile.TileContext(nc) as tc, ExitStack() as ctx:
        wpool = ctx.enter_context(tc.tile_pool(name="wpool", bufs=1))
        xpool = ctx.enter_context(tc.tile_pool(name="xpool", bufs=2))
        bigpool = ctx.enter_context(tc.tile_pool(name="bigpool", bufs=1))
        epool = ctx.enter_context(tc.tile_pool(name="epool", bufs=34))
        spool = ctx.enter_context(tc.tile_pool(name="spool", bufs=6))
        pp_acc = ctx.enter_context(tc.tile_pool(name="pp_acc", bufs=2, space="PSUM"))
        pp_sc = ctx.enter_context(tc.tile_pool(name="pp_sc", bufs=2, space="PSUM"))
        pp_ctx = ctx.enter_context(tc.tile_pool(name="pp_ctx", bufs=1, space="PSUM"))

        # ---- constants into SBUF ----
        wq_s = wpool.tile([P, KC2, KC, 2, P], F8, tag="wq")
        wk_s = wpool.tile([P, KC2, KC, 2, P], F8, tag="wk")
        wv_s = wpool.tile([P, KC, 2, 512], F8, tag="wv")
        wo_s = wpool.tile([P, KC2, 2, D], F8, tag="wo")
        bq_s = wpool.tile([P, KC2], F32, tag="bq")
        bk_s = wpool.tile([P, KC2], F32, tag="bk")
        pad_s = wpool.tile([P, S // P], F32, tag="pad")
        tri_s = wpool.tile([P, QW], BF16, tag="tri")
        eye_s = wpool.tile([P, P], BF16, tag="eye")
        # touch Exp once at t=0 so the ~1.3us ACT table load happens inside
        # the startup DMA shadow, not at the first real softmax
        warm_s = wpool.tile([P, 1], F32, tag="warm")
        nc.vector.memset(warm_s[:], 0.0)
        nc.scalar.activation(warm_s[:], warm_s[:],
                             mybir.ActivationFunctionType.Exp, scale=1.0)

        # ---- big persistent activations ----
        # q digits [pairdims, pair, {A,B}, q]; k digits [pairdims, pair, {K1,K2}, k]
        qd_s = bigpool.tile([P, KC2, 2, S], F8, tag="qd")
        kd_s = bigpool.tile([P, KC2, 2, S], F8, tag="kd")
        v_s = bigpool.tile([P, S // P, 8, HD + 1], BF16, tag="v")  # [k, ktile, h, hd|1]
        nc.vector.memset(v_s[:, :, :, HD:HD + 1], 1.0)
        # ctx digits [pairdims, pair, {c8,cr}, q]
        cd_s = bigpool.tile([P, KC2, 2, S], F8, tag="cd")

        def load_xt(st):
            xt_t = xpool.tile([P, KC, 2, QW], F8, tag="xt")
            nc.sync.dma_start(xt_t[:], xt[:, st, :, :, :])
            return xt_t

        def qkv_tiles(st, xt_t, parts="qkv", on_act=False, ms=None,
                      dig_dve=False):
            """Thunks projecting tokens [st*512, (st+1)*512): Q stripe st,
            K/V k-tiles 4*st..4*st+3, all fp8 DoubleRow 3-term. Each
            (matrix, m) splits into 3 PE emission units (~427ns each):
            main(4 DR), crossA(4 DR), crossB(4 DR)+psum->digit handoff."""
            ssl = slice(st * QW, (st + 1) * QW)
            thunks = []
            state = {}

            def qk_unit(w_s, b_s, dst, m, part, key):
                # part 0: main pairs; part 1: cross c=0..3; part 2: cross c=4..7
                # then qbf (DVE) + digit A (pool) + digit B (pool)
                if part == 0:
                    state[key] = pp_acc.tile([P, QW], F32, tag="acc",
                                             name="acc_ps")
                    ps = state[key]
                    for c in (0, 2, 4, 6):
                        nc.tensor.matmul(
                            ps[:], lhsT=w_s[:, m, c:c + 2, 1, :],
                            rhs=xt_t[:, c:c + 2, 0, :],
                            start=(c == 0), stop=False, perf_mode=DR)
                else:
                    ps = state[key]
                    for c in range(4 * (part - 1), 4 * part):
                        nc.tensor.matmul(
                            ps[:], lhsT=w_s[:, m, c, 0:2, :],
                            rhs=xt_t[:, c, 0:2, :],
                            start=False, stop=(c == KC - 1), perf_mode=DR)
                    if part == 2:
                        qbf = spool.tile([P, QW], BF16, tag="qbf",
                                         name="qbf", bufs=3)
                        nc.vector.tensor_scalar_add(qbf[:], ps[:],
                                                    b_s[:, m:m + 1])
                        del state[key]
                        # DVE for the startup digits (Pool's Q7 launches
                        # would gate the first scores), Pool afterwards
                        eng = nc.vector if dig_dve else nc.gpsimd
                        eng.tensor_scalar_mul(
                            dst[:, m, 0, ssl], qbf[:], 0.5)
                        eng.tensor_tensor(
                            dst[:, m, 1, ssl], qbf[:], dst[:, m, 0, ssl],
                            mybir.AluOpType.subtract)

            def v_unit(sub, part, key):
                subsl = slice(sub * P, (sub + 1) * P)
                if part == 0:
                    state[key] = pp_acc.tile([P, 8, HD], F32, tag="acc",
                                             name="acc_ps")
                    ps = state[key]
                    for c in (0, 2, 4, 6):
                        nc.tensor.matmul(
                            ps[:], lhsT=xt_t[:, c:c + 2, 0, subsl],
                            rhs=wv_s[:, c:c + 2, 1, :],
                            start=(c == 0), stop=False, perf_mode=DR)
                else:
                    ps = state[key]
                    for c in range(4 * (part - 1), 4 * part):
                        nc.tensor.matmul(
                            ps[:], lhsT=xt_t[:, c, 0:2, subsl],
                            rhs=wv_s[:, c, 0:2, :],
                            start=False, stop=(c == KC - 1), perf_mode=DR)
                    if part == 2:
                        nc.vector.tensor_copy(
                            out=v_s[:, st * 4 + sub, :, 0:HD], in_=ps[:])
                        del state[key]

            plan = []
            if "q" in parts:
                plan.append(("q", wq_s, bq_s, qd_s))
            if "k" in parts:
                plan.append(("k", wk_s, bk_s, kd_s))
            for pn, w_s, b_s, dst in plan:
                for m in (range(KC2) if ms is None else ms):
                    for part in range(3):
                        tag = (st, pn, m) if part == 2 else None
                        thunks.append(
                            (427, lambda w_s=w_s, b_s=b_s, dst=dst, m=m,
                             part=part, key=(pn, m):
                             qk_unit(w_s, b_s, dst, m, part, key), tag))
            if "v" in parts:
                for sub in range(4):
                    for part in range(3):
                        tag = (st, "v", sub) if part == 2 else None
                        thunks.append((427, lambda sub=sub, part=part,
                                       key=("v", sub): v_unit(sub, part, key),
                                       tag))
            return thunks

        def qkv_stage(st, xt_t, parts="qkv", ms=None, dig_dve=False):
            for _, t, tag in qkv_tiles(st, xt_t, parts, ms=ms,
                                       dig_dve=dig_dve):
                t()
                if tag:
                    done_tags.add(tag)

        from collections import deque
        fillers = deque()   # (ns, thunk, tag-or-None)
        done_tags = set()

        def pop_filler():
            ns, t, tag = fillers.popleft()
            t()
            if tag:
                done_tags.add(tag)
            return ns

        def ensure(tag):
            while tag not in done_tags and fillers:
                pop_filler()

        def drain():
            while fillers:
                pop_filler()

        attn_state = {}

        def attn_ph1(stripe, pr, j):
            """Scores/exp/mask for one (pair, k-tile); e kept in SBUF.
            One DR matmul per head: slots (K1,A)+(K2,B) ~= q16.k16/2."""
            es = attn_state[(stripe, pr)]
            m = j
            ksl = slice(m * P, (m + 1) * P)
            diag = m >= 4 * stripe
            # within a diagonal k-tile of shift t = m-4s, queries below
            # t*128 are entirely masked -- compute only the valid sub-range
            off = (m - 4 * stripe) * P if diag else 0
            w = QW - off
            qsub = slice(stripe * QW + off, (stripe + 1) * QW)
            sc = pp_sc.tile([P, 2 * QW], F32, tag="sc")
            for q_i in range(2):
                lo = q_i * HD
                nc.tensor.matmul(
                    sc[:, q_i * QW + off:(q_i + 1) * QW],
                    lhsT=kd_s[lo:lo + HD, pr, 0:2, ksl],
                    rhs=qd_s[lo:lo + HD, pr, 0:2, qsub],
                    start=True, stop=True, perf_mode=DR,
                    tile_position=(lo, 0))
            e = epool.tile([P, 2 * QW], BF16, tag="e")
            es.append(e)
            if off >= 256:
                # narrow diagonal exps: two short instructions beat one
                # full-width one once off >= 256 (ACT is the critical engine)
                for q_i in range(2):
                    esl = slice(q_i * QW + off, (q_i + 1) * QW)
                    nc.scalar.activation(e[:, esl], sc[:, esl],
                                         mybir.ActivationFunctionType.Exp,
                                         bias=pad_s[:, m:m + 1],
                                         scale=ESCALE)
            else:
                # full-width exp: any masked query columns hold garbage
                # (stale PSUM) but are never read downstream
                nc.scalar.activation(e[:], sc[:],
                                     mybir.ActivationFunctionType.Exp,
                                     bias=pad_s[:, m:m + 1],
                                     scale=ESCALE)
            if diag:
                # only the 128-wide diagonal query block needs the triangle;
                # beyond it tri is all-ones (no-op)
                for q_i in range(2):
                    esl = slice(q_i * QW + off, q_i * QW + off + P)
                    nc.vector.tensor_tensor(
                        e[:, esl], e[:, esl], tri_s[:, 0:P],
                        mybir.AluOpType.mult)

        lastcbf = {}

        def attn_ph2_parts(stripe, pr, last=False):
            """Post-phase-1 work for a pair. Returns {"region": fn,
            "units": [...]}. Normal pairs: units include the 8 ctx region
            bursts and end with the fp8 digit split of the transposed ctx.
            last=True: regions are called eagerly by the caller inside the
            ph1 j-loop, norms read PSUM directly (skip craw), and the digit
            split is skipped -- the tail oproj closes consume the bf16
            transpose staging buffer instead."""
            qsl = slice(stripe * QW, (stripe + 1) * QW)
            es = attn_state[(stripe, pr)]
            nkt = 4 * stripe + 4
            st = {}

            def region(q_i, qb):
                if (q_i, qb) == (0, 0):
                    st["ctx_ps"] = pp_ctx.tile([P, 2, QW], F32, tag="ctx",
                                               name="ctx_ps")
                ctx_ps = st["ctx_ps"]
                h = 2 * pr + q_i
                js = list(range(0, 4 * stripe + qb + 1))
                for i, j in enumerate(js):
                    nc.tensor.matmul(
                        ctx_ps[:, q_i, qb * (HD + 1):(qb + 1) * (HD + 1)],
                        lhsT=es[j][:, q_i * QW + qb * P:q_i * QW + (qb + 1) * P],
                        rhs=v_s[:, j, h, :],
                        start=(i == 0), stop=(i == len(js) - 1),
                        skip_group_check=True)

            def recip():
                st["rden"] = spool.tile([P, 2, 4], F32, tag="rden",
                                        name="rden", bufs=2)
                nc.vector.reciprocal(
                    st["rden"][:],
                    st["ctx_ps"][:, :, HD:4 * (HD + 1):HD + 1])

            def recip_qb(qb):
                if "rden" not in st:
                    st["rden"] = spool.tile([P, 2, 4], F32, tag="rden",
                                            name="rden", bufs=2)
                nc.vector.reciprocal(
                    st["rden"][:, :, qb:qb + 1],
                    st["ctx_ps"][:, :, qb * (HD + 1) + HD:
                                 qb * (HD + 1) + HD + 1])

            def copyraw():
                # one bf16 copy frees the 2-bank ctx slot immediately; the
                # per-region normalizes then read SBUF at 2x DVE rate
                st["craw"] = spool.tile([P, 2, QW], BF16, tag="craw",
                                        name="craw", bufs=2)
                nc.vector.tensor_copy(out=st["craw"][:], in_=st["ctx_ps"][:])
                del st["ctx_ps"]
                attn_state[(stripe, pr)] = []  # release e tiles

            def norm(q_i, qb):
                if last:
                    src = st["ctx_ps"][:, q_i, qb * (HD + 1):qb * (HD + 1) + HD]
                else:
                    src = st["craw"][:, q_i, qb * (HD + 1):qb * (HD + 1) + HD]
                nc.vector.tensor_scalar_mul(
                    st["ctxc"][:, (q_i * 4 + qb) * HD:(q_i * 4 + qb + 1) * HD],
                    src,
                    st["rden"][:, q_i, qb:qb + 1])

            def mkctxc():
                st["ctxc"] = spool.tile([P, QW], BF16, tag="ctxc",
                                        name="ctxc", bufs=2)

            def transpose():
                st["pt"] = pp_ctx.tile([P, 2, QW], F32, tag="ctx", name="pt")
                for q_i in range(2):
                    for qb in range(4):
                        nc.tensor.matmul(
                            st["pt"][q_i * HD:(q_i + 1) * HD, 0,
                                     qb * P:(qb + 1) * P],
                            lhsT=st["ctxc"][:, (q_i * 4 + qb) * HD:
                                            (q_i * 4 + qb + 1) * HD],
                            rhs=eye_s[:],
                            start=True, stop=True,
                            skip_group_check=True)

            def ptcopy():
                # stage psum -> bf16, then split to fp8 digits on Pool
                st["cbf"] = spool.tile([P, QW], BF16, tag="cbf",
                                       name="cbf", bufs=2)
                nc.vector.tensor_copy(out=st["cbf"][:], in_=st["pt"][:, 0, :])
                del st["pt"]
                if last:
                    lastcbf[0] = st["cbf"]
                    del attn_state[(stripe, pr)]

            def cdig():
                nc.gpsimd.tensor_scalar_mul(cd_s[:, pr, 0, qsl],
                                            st["cbf"][:], 1.0)
                nc.gpsimd.tensor_tensor(cd_s[:, pr, 1, qsl], st["cbf"][:],
                                        cd_s[:, pr, 0, qsl],
                                        mybir.AluOpType.subtract)
                del attn_state[(stripe, pr)]

            if last:
                # per-qb finish: reciprocal + both heads' normalizes as soon
                # as that qb's denominator lands, inside the exp shadow
                def early_finish(qb):
                    if qb == 0:
                        mkctxc()
                    recip_qb(qb)
                    norm(0, qb)
                    norm(1, qb)

                units = [(0, lambda: st.pop("ctx_ps")),
                         (427, transpose), (0, ptcopy)]
                return {"region": region, "early_finish": early_finish,
                        "units": units}
            units = [(27 * (4 * stripe + qb + 1),
                      lambda q_i=q_i, qb=qb: region(q_i, qb))
                     for q_i in range(2) for qb in range(4)]
            units += [(0, recip), (0, copyraw), (0, mkctxc)]
            units += [(0, lambda q_i=q_i, qb=qb: norm(q_i, qb))
                      for q_i in range(2) for qb in range(4)]
            units += [(427, transpose), (0, ptcopy), (0, cdig)]
            return {"region": region, "units": units}

        def attn_ph2_units(stripe, pr):
            return attn_ph2_parts(stripe, pr)["units"]

        pending = deque()  # ph2 units of the previously finished pair
        carry = [0]        # un-met PE deficit banked across k-tiles

        def attn_pair(stripe, pr, budget_ns=600, last=False):
            """Phase 1 j-loop for one pair, interleaving ph2 units of the
            previous pair (and fillers) between k-tiles. The exp costs
            ~1038ns/k-tile vs ~213ns of DR scores, so ~800ns of PE filler
            per k-tile keeps the PE dense; unmet remainder banks forward.
            last=True runs this pair's ctx regions eagerly inside the j-loop
            and its normalize/transpose chain immediately after, shortening
            the post-exp tail."""
            nkt = 4 * stripe + 4
            attn_state[(stripe, pr)] = []
            parts = attn_ph2_parts(stripe, pr, last=True) if last else None
            for j in range(nkt):
                attn_ph1(stripe, pr, j)
                if last and j >= 4 * stripe:
                    qb = j - 4 * stripe
                    if qb == 0:
                        # the previous pair's ph2 must be fully emitted
                        # before this pair claims the single ctx PSUM slot
                        while pending:
                            pending.popleft()[1]()
                    parts["region"](0, qb)
                    parts["region"](1, qb)
                    parts["early_finish"](qb)
                budget = carry[0] + budget_ns
                while budget > 100:
                    if pending:
                        ns, t = pending.popleft()
                        t()
                    elif fillers:
                        ns = pop_filler()
                    else:
                        break
                    budget -= max(ns, 50)
                carry[0] = min(max(budget, 0), B_CARRY)
            if last:
                for _, t in parts["units"]:
                    t()
            else:
                pending.extend(attn_ph2_units(stripe, pr))

        def oproj_tiles(t8, alt=False, quarters=False):
            """Thunks for one 128-token output block: per 512-dout half,
            6 DR instrs (2 main chunk-pairs + 4 cross), ~640ns, or three
            ~213ns thirds (quarters=True)."""
            osl = slice(t8 * P, (t8 + 1) * P)
            state = {}

            def mm_main(ps, cs, start):
                for c in cs:
                    nc.tensor.matmul(ps[:], lhsT=cd_s[:, c:c + 2, 0, osl],
                                     rhs=wo_s[:, c:c + 2, 1,
                                              state["dsl"]],
                                     start=(start and c == cs[0]), stop=False,
                                     perf_mode=DR)

            def mm_cross(ps, kcs, stop):
                for kc in kcs:
                    nc.tensor.matmul(ps[:], lhsT=cd_s[:, kc, 0:2, osl],
                                     rhs=wo_s[:, kc, 0:2, state["dsl"]],
                                     start=False, stop=(stop and kc == kcs[-1]),
                                     perf_mode=DR)

            def fin(dt, ps):
                # both 512-halves stage into one tile; a single [128, 1024]
                # DMA per token block halves HWDGE/semaphore traffic
                if dt == 0:
                    state["ob"] = spool.tile([P, 2, QW], BF16, tag="outsb",
                                             name="ob", bufs=3)
                ob = state["ob"]
                nc.vector.tensor_copy(out=ob[:, dt, :], in_=ps[:])
                if dt == 1:
                    (nc.scalar if alt else nc.sync).dma_start(
                        out[osl, :], ob[:])
                    del state["ob"]

            def whole(dt):
                state["dsl"] = slice(dt * QW, (dt + 1) * QW)
                if alt and dt == 1:
                    ps = pp_sc.tile([P, 2 * QW], F32, tag="sc",
                                    name="oproj_ps")[:, 0:QW]
                else:
                    ps = pp_acc.tile([P, QW], F32, tag="acc", name="oproj_ps")
                mm_main(ps, [0, 2], True)
                mm_cross(ps, range(KC2), True)
                fin(dt, ps)

            def qopen(dt):
                state["dsl"] = slice(dt * QW, (dt + 1) * QW)
                state[dt] = pp_acc.tile([P, QW], F32, tag="acc",
                                        name="oproj_ps")
                mm_main(state[dt], [0, 2], True)
                mm_cross(state[dt], [0, 1], False)

            def qclose(dt):
                state["dsl"] = slice(dt * QW, (dt + 1) * QW)
                ps = state.pop(dt)
                mm_cross(ps, [2, 3], True)
                fin(dt, ps)

            if quarters:
                return [lambda dt=dt, f=f: f(dt)
                        for dt in range(2) for f in (qopen, qclose)]
            return [lambda dt=dt: whole(dt) for dt in range(2)]

        def oproj(t8, alt=False):
            for t in oproj_tiles(t8, alt):
                t()

        heldpart = {}

        def oproj_openA(t8, dt):
            """Pairs 0/1 products of a final oproj tile -- legal as soon as
            cdig(3,1) has popped, i.e. inside pair (3,2)'s window where the
            PE otherwise starves. Staged to SBUF bf16; the close replays
            pair 2 + the bf16 last-pair products and adds this partial."""
            ps = pp_acc.tile([P, QW], F32, tag="acc", name="oproj_ps")
            dsl = slice(dt * QW, (dt + 1) * QW)
            osl = slice(t8 * P, (t8 + 1) * P)
            nc.tensor.matmul(ps[:], lhsT=cd_s[:, 0:2, 0, osl],
                             rhs=wo_s[:, 0:2, 1, dsl],
                             start=True, stop=False, perf_mode=DR)
            for kc in range(2):
                nc.tensor.matmul(ps[:], lhsT=cd_s[:, kc, 0:2, osl],
                                 rhs=wo_s[:, kc, 0:2, dsl],
                                 start=False, stop=(kc == 1), perf_mode=DR)
            part = spool.tile([P, QW], BF16, tag="opart",
                              name=f"opart{t8}_{dt}", bufs=8)
            nc.vector.tensor_copy(out=part[:], in_=ps[:])
            heldpart[(t8, dt)] = part

        heldob = {}

        heldps = {}

        def oproj_close(t8, dt):
            part = heldpart.pop((t8, dt))
            cbf = lastcbf[0]
            tloc = slice((t8 - 12) * P, (t8 - 11) * P)
            ps = pp_acc.tile([P, QW], F32, tag="acc", name="oproj_ps")
            dsl = slice(dt * QW, (dt + 1) * QW)
            osl = slice(t8 * P, (t8 + 1) * P)
            nc.tensor.matmul(ps[:], lhsT=cd_s[:, 2, 0:2, osl],
                             rhs=wo_s[:, 2, 0:2, dsl],
                             start=True, stop=False, perf_mode=DR)
            nc.tensor.matmul(ps[:], lhsT=cd_s[:, 2, 0, osl],
                             rhs=wo_s[:, 2, 1, dsl],
                             start=False, stop=False)
            nc.tensor.matmul(ps[:], lhsT=cbf[:, tloc],
                             rhs=wo_s[:, 3, 1, dsl],
                             start=False, stop=False)
            nc.tensor.matmul(ps[:], lhsT=cbf[:, tloc],
                             rhs=wo_s[:, 3, 0, dsl],
                             start=False, stop=True)
            if dt == 0:
                heldob[t8] = spool.tile([P, 2, QW], BF16, tag="outsb",
                                        name="ob", bufs=3)
            ob = heldob[t8]
            nc.vector.tensor_tensor(ob[:, dt, :], ps[:], part[:],
                                    mybir.AluOpType.add)
            # per-half DMA: each half ships as soon as its add lands
            (nc.sync if t8 % 2 else nc.scalar).dma_start(out[osl, dsl],
                                                         ob[:, dt, :])
            if dt == 1:
                del heldob[t8]

        # ---- schedule ----
        # Startup DMAs: the minimal set for pair (0,0) first (xt0, m0 of
        # Wq/Wk, biases, pad, tri), then attention starts while the rest of
        # the weights stream in and the remaining stage work rides fillers.
        xt0 = xpool.tile([P, KC, 2, QW], F8, tag="xt")
        nc.scalar.dma_start(xt0[:, 0:2, :, :], xt[:, 0, 0:2, :, :])
        nc.sync.dma_start(xt0[:, 4:6, :, :], xt[:, 0, 4:6, :, :])
        # PE warmup on a zeroed tile: keeps the PE continuously busy through
        # the startup DMA shadow so the first real matmuls run at full clock
        # (the cost model's p-state ramp needs ~3us of uninterrupted work)
        wmm = wpool.tile([P, QW], BF16, tag="wmm")
        nc.gpsimd.memset(wmm[:], 0.0)
        wps = pp_sc.tile([P, 2 * QW], F32, tag="sc", name="warm_ps")
        for i in range(8):
            nc.tensor.matmul(wps[:, 0:QW], lhsT=wmm[:, 0:P], rhs=wmm[:],
                             start=(i == 0), stop=(i == 7))
        nc.scalar.dma_start(xt0[:, 2:4, :, :], xt[:, 0, 2:4, :, :])
        nc.sync.dma_start(wk_s[:, 0, :, :, :], wk[:, 0, :, :, :])
        nc.scalar.dma_start(xt0[:, 6:8, :, :], xt[:, 0, 6:8, :, :])
        nc.sync.dma_start(wq_s[:, 0, :, :, :], wq[:, 0, :, :, :])
        nc.scalar.dma_start(bk_s[:], bkp[:])
        nc.sync.dma_start(bq_s[:], bqp[:])
        nc.scalar.dma_start(tri_s[:], tri[:])
        nc.sync.dma_start(pad_s[:], pad[:])
        # q digits on DVE, k digits on Pool: the two first-pair digit chains
        # run on different engines in parallel
        qkv_stage(0, xt0, parts="q", ms=[0], dig_dve=True)
        qkv_stage(0, xt0, parts="k", ms=[0])
        nc.sync.dma_start(wv_s[:, 0:4, :, :], wv[:, 0:4, :, :])
        nc.scalar.dma_start(wv_s[:, 4:8, :, :], wv[:, 4:8, :, :])
        xt1 = load_xt(1)
        nc.sync.dma_start(wq_s[:, 1, :, :, :], wq[:, 1, :, :, :])
        nc.scalar.dma_start(wk_s[:, 1, :, :, :], wk[:, 1, :, :, :])
        nc.sync.dma_start(wq_s[:, 2, :, :, :], wq[:, 2, :, :, :])
        nc.scalar.dma_start(wk_s[:, 2, :, :, :], wk[:, 2, :, :, :])
        nc.sync.dma_start(wq_s[:, 3, :, :, :], wq[:, 3, :, :, :])
        nc.scalar.dma_start(wk_s[:, 3, :, :, :], wk[:, 3, :, :, :])
        nc.scalar.dma_start(eye_s[:], eye[:])

        # stripe 0/1 pair interleave: stripe 0 alone supplies too little exp
        # work to keep ACT busy through the projection-heavy opening, so
        # stripe-1 pairs (2x the exp volume) run in between
        fillers.extend(qkv_tiles(1, xt1, parts="q", ms=[0]))
        fillers.extend(qkv_tiles(1, xt1, parts="k", ms=[0]))
        fillers.extend(qkv_tiles(0, xt0, parts="v"))
        fillers.extend(qkv_tiles(0, xt0, parts="q", ms=[1]))
        fillers.extend(qkv_tiles(0, xt0, parts="k", ms=[1]))
        fillers.extend(qkv_tiles(1, xt1, parts="v"))
        for m in (1, 2, 3):
            sts = (1,) if m == 1 else (0, 1)
            for stq in sts:
                xtt = xt0 if stq == 0 else xt1
                fillers.extend(qkv_tiles(stq, xtt, parts="q", ms=[m]))
                fillers.extend(qkv_tiles(stq, xtt, parts="k", ms=[m]))
        nc.sync.dma_start(wo_s[:], wo[:])
        attn_pair(0, 0, budget_ns=900)
        ensure((1, "q", 0))
        ensure((1, "k", 0))
        ensure((0, "v", 3))
        attn_pair(1, 0, budget_ns=B_EARLY)
        ensure((0, "q", 1))
        ensure((0, "k", 1))
        ensure((1, "v", 3))
        attn_pair(0, 1, budget_ns=B_EARLY)
        ensure((1, "q", 1))
        ensure((1, "k", 1))
        attn_pair(1, 1, budget_ns=B_EARLY)
        ensure((0, "q", 2))
        ensure((0, "k", 2))
        attn_pair(0, 2, budget_ns=B_EARLY)
        ensure((1, "q", 2))
        ensure((1, "k", 2))
        attn_pair(1, 2, budget_ns=B_EARLY)
        ensure((0, "q", 3))
        ensure((0, "k", 3))
        attn_pair(0, 3, budget_ns=B_EARLY)
        ensure((1, "q", 3))
        ensure((1, "k", 3))
        xt2 = load_xt(2)

        def extend_stage_qk(st, xt_t):
            for m in range(KC2):
                fillers.extend(qkv_tiles(st, xt_t, parts="q", ms=[m]))
                fillers.extend(qkv_tiles(st, xt_t, parts="k", ms=[m]))

        extend_stage_qk(2, xt2)
        fillers.extend(qkv_tiles(2, xt2, parts="v"))
        attn_pair(1, 3, budget_ns=B_EARLY)
        xt3 = load_xt(3)
        extend_stage_qk(3, xt3)
        for pr in range(4):
            ensure((2, "q", pr))
            ensure((2, "k", pr))
            if pr == 1:
                ensure((2, "v", 3))
            attn_pair(2, pr, budget_ns=B_MID)
        # V(3) is safe here -- first needed by the ctx regions of pair (3,0),
        # which only run during ph1(3,1). oproj of stripe-2 blocks becomes
        # legal once pair (2,3)'s pending units pop at the start of (3,0).
        fillers.extend(qkv_tiles(3, xt3, parts="v"))
        for t8 in range(0, 11):
            fillers.extend((213, t, None)
                           for t in oproj_tiles(t8, quarters=True))
        for pr in range(2):
            ensure((3, "q", pr))
            ensure((3, "k", pr))
            if pr == 1:
                ensure((3, "v", 3))
            attn_pair(3, pr, budget_ns=B_S3)
        # openA (pairs 0/1 products) of the final oproj tiles becomes legal
        # once cdig(3,1) pops -- feed it to pair (3,2)'s otherwise-starved
        # PE, together with the remaining stripe-2 oproj tile
        ensure((3, "q", 2))
        ensure((3, "k", 2))
        fillers.extend((213, t, None)
                       for t in oproj_tiles(11, quarters=True))
        for t8 in range(12, 16):
            fillers.append((320, (lambda t8=t8: oproj_openA(t8, 0)), None))
            fillers.append((320, (lambda t8=t8: oproj_openA(t8, 1)), None))
        attn_pair(3, 2, budget_ns=B_S3)
        ensure((3, "q", 3))
        ensure((3, "k", 3))
        attn_pair(3, 3, budget_ns=B_LAST, last=True)
        while pending:
            pending.popleft()[1]()
        drain()
        for t8 in range(12, 16):
            oproj_close(t8, 0)
            oproj_close(t8, 1)

    nc.compile()
    return nc


def _dig(a):
    """two-digit e4m3 split along a new axis: returns np [..., 2] fp8"""
    hi = a.astype(NPE4)
    lo = (a - hi.astype(np.float32)).astype(NPE4)
    return hi, lo


def _core_inputs(c, x, padding_mask, Wq, bq, Wk, bk, Wv, bv, Wo, bo):
    b, hh = c // 2, c % 2
    hsl = slice(hh * 512, (hh + 1) * 512)

    xb = np.ascontiguousarray(
        x[b].T.reshape(KC, P, S).transpose(1, 0, 2)).astype(np.float32)
    x8, xr = _dig(xb)
    # [P, KC, 2, S] -> stripe-major [P, NS, KC, 2, QW]
    xt = np.stack([x8, xr], axis=2).reshape(P, KC, 2, NS, QW)
    xt = np.ascontiguousarray(xt.transpose(0, 3, 1, 2, 4))

    def wl(Wh):  # [512 out, 1024 in] -> m-major [P, KC2, KC, 2, 128] {Wr,W8}
        w = np.ascontiguousarray(
            Wh.T.reshape(KC, P, 512).transpose(1, 0, 2)).astype(np.float32)
        w8, wr = _dig(WS * w)
        st = np.stack([wr, w8], axis=2)          # [P, KC, 2, 512]
        st = st.reshape(P, KC, 2, KC2, P).transpose(0, 3, 1, 2, 4)
        return np.ascontiguousarray(st)

    def wvl(Wh):  # [512 out, 1024 in] -> chunk-major [P, KC, 2, 512] {Wr,W8}
        w = np.ascontiguousarray(
            Wh.T.reshape(KC, P, 512).transpose(1, 0, 2)).astype(np.float32)
        w8, wr = _dig(WS * w)
        return np.ascontiguousarray(np.stack([wr, w8], axis=2))

    wob = np.ascontiguousarray(
        Wo[:, hsl].T.reshape(KC2, P, D).transpose(1, 0, 2)).astype(np.float32)
    wo8, wor = _dig(WS * wob)
    wol = np.ascontiguousarray(np.stack([wor, wo8], axis=2))

    bqp = np.ascontiguousarray(
        WS * bq[hsl].reshape(KC2, P).T).astype(np.float32)
    bkp = np.ascontiguousarray(
        WS * bk[hsl].reshape(KC2, P).T).astype(np.float32)

    padb = np.where(padding_mask[b].reshape(S // P, P).T, 0.0,
                    NEG).astype(np.float32)
    padb = np.ascontiguousarray(padb)

    kk = np.arange(P)[:, None]
    uu = np.arange(QW)[None, :]
    trib = np.ascontiguousarray((kk <= uu).astype(NPBF16))

    return {"xt": xt, "wq": wl(Wq[hsl]), "wk": wl(Wk[hsl]), "wv": wvl(Wv[hsl]),
            "wo": wol, "bqp": bqp, "bkp": bkp, "pad": padb, "tri": trib,
            "eye": np.eye(P, dtype=NPBF16)}


_NC_CACHE = {}


def kernel(x, padding_mask, Wq, bq, Wk, bk, Wv, bv, Wo, bo):
    x = np.asarray(x, np.float32)
    padding_mask = np.asarray(padding_mask, bool)
    args = [np.asarray(a, np.float32) for a in (Wq, bq, Wk, bk, Wv, bv, Wo, bo)]

    if "nc" not in _NC_CACHE:
        _NC_CACHE["nc"] = _build()
    nc = _NC_CACHE["nc"]

    in_maps = [_core_inputs(c, x, padding_mask, *args) for c in range(8)]

    trace = bool(int(os.environ.get("KERNEL_TRACE", "0")))
    try:
        res = run_bass_kernel_spmd(nc, in_maps, core_ids=list(range(8)), trace=trace)
    except ModuleNotFoundError:
        res = run_bass_kernel_spmd(nc, in_maps, core_ids=list(range(8)))
    if trace and res.exec_time_ns is not None:
        print(f"HW exec time: {res.exec_time_ns} ns")
        _NC_CACHE["exec_time_ns"] = res.exec_time_ns

    Wo_, bv_, bo_ = args[6], args[5], args[7]
    btot = (bo_ + Wo_ @ bv_).astype(np.float32)
    descale = 1.0 / (WS * WS)
    full = np.empty((B, S, D), np.float32)
    for b in range(B):
        full[b] = ((res.results[2 * b]["out"].astype(np.float32)
                    + res.results[2 * b + 1]["out"].astype(np.float32))
                   * descale + btot)
    return full


if __name__ == "__main__":
    rng = np.random.default_rng(0)
    x = rng.standard_normal((B, S, D), dtype=np.float32)
    lengths = rng.integers(S // 2, S + 1, size=(B,))
    pm = np.arange(S)[None, :] < lengths[:, None]
    std = 0.02
    ws = {n: (rng.standard_normal((D, D), dtype=np.float32) * std)
          for n in ("Wq", "Wk", "Wv", "Wo")}
    z = np.zeros((D,), np.float32)
    out = kernel(x, pm, ws["Wq"], z, ws["Wk"], z, ws["Wv"], z, ws["Wo"], z)
    print(out.shape, out.dtype, np.abs(out).mean())


# revision 5
# speedup vs baseline: 1.0183x; 1.0181x over previous
"""Causal MHA (B=4, S=2048, D=1024, H=16) on 8 TRN2 cores, head-parallel,
fp8 DoubleRow edition.

Core c = (batch b=c//2, head-half hh=c%2). Same schedule skeleton as the
bf16 baseline, but every projection matmul runs fp8e4m3 DoubleRow:

- QKV/O projections: 3-term compensated digits (x = x8+xr, 16W = W8+Wr,
  dropping the xr*Wr term) -> bf16-level accuracy at 6/8 the bf16 PE cost.
  Digit pairs pack into DR slots: main instrs pair (W8_c, W8_c+1) x
  (x8_c, x8_c+1); cross instrs pair (Wr_c, W8_c) x (x8_c, xr_c).
- Scores: twin-sample split q16 = A+B (A = e4(q16/2), B = e4(q16-A)),
  k16 = K1+K2; one DR instr per head per k-tile computes K1.A + K2.B
  ~= q16.k16/2 with ~2.5% rms error (vs 3.6% single-digit), at HALF the
  bf16 score cost. exp scale absorbs the 2/(256*8) factor.
- ctx + transpose stay bf16 (e-quantization to fp8 would break the 2e-2
  gate). ctx values carry a 16x scale (v = x@(16Wv)); the oproj digits
  c8/cr quantize the 16x-scaled ctx (good e4m3 range), and the host
  divides partial outputs by 256.

fp8 digit production rides the idle GpSimd(Pool) engine + DVE slack.
"""

import os
import sys

sys.path.insert(0, "/opt/trn_rl_repo")

import numpy as np
import ml_dtypes

import concourse.bass as bass
import concourse.bacc as bacc
import concourse.tile as tile
from concourse import mybir
from concourse.bass_utils import run_bass_kernel_spmd

B, S, D, H = 4, 2048, 1024, 16
HD = D // H  # 64
P = 128
KC = D // P   # 8 contraction chunks for QKV projections
KC2 = 4       # contraction chunks for O projection (512 dims)
QW = 512      # query stripe width
NS = S // QW  # 4 stripes
NEG = -1e30
BF16 = mybir.dt.bfloat16
F32 = mybir.dt.float32
F8 = mybir.dt.float8e4
NPBF16 = ml_dtypes.bfloat16
NPE4 = ml_dtypes.float8_e4m3
WS = 16.0           # host weight scale
B_EARLY = 600   # filler budget/k-tile: stripe 0/1 interleave
B_MID = 700     # stripe 1 tail + stripe 2
B_S3 = 900      # stripe 3 pairs 0-2
B_LAST = 1300   # final pair
B_CARRY = 1500  # carry cap
WARM_N = 8      # PE warmup matmuls
B_P00 = 700     # first pair budget
ESCALE = 2.0 / (WS * WS * 8.0)   # exp scale: twin-slot 2x / (16*16 * sqrt(hd))
DR = mybir.MatmulPerfMode.DoubleRow


def _build():
    nc = bacc.Bacc()

    # x digits, stripe-major: [indim-part, stripe, chunk, {x8,xr}, tok]
    xt = nc.declare_dram_parameter("xt", [P, NS, KC, 2, QW], F8, isOutput=False)
    # Wq/Wk digits, m-major: [indim-part, m, chunk, {Wr,W8}, 128 outdim]
    wq = nc.declare_dram_parameter("wq", [P, KC2, KC, 2, P], F8, isOutput=False)
    wk = nc.declare_dram_parameter("wk", [P, KC2, KC, 2, P], F8, isOutput=False)
    # Wv digits, chunk-major: [indim-part, chunk, {Wr,W8}, 512 outdim]
    wv = nc.declare_dram_parameter("wv", [P, KC, 2, 512], F8, isOutput=False)
    # Wo digits: [dh-part, chunk(=pair), {Wor,Wo8}, dout]
    wo = nc.declare_dram_parameter("wo", [P, KC2, 2, D], F8, isOutput=False)
    bqp = nc.declare_dram_parameter("bqp", [P, KC2], F32, isOutput=False)
    bkp = nc.declare_dram_parameter("bkp", [P, KC2], F32, isOutput=False)
    pad = nc.declare_dram_parameter("pad", [P, S // P], F32, isOutput=False)
    tri = nc.declare_dram_parameter("tri", [P, QW], BF16, isOutput=False)
    eye = nc.declare_dram_parameter("eye", [P, P], BF16, isOutput=False)
    out = nc.declare_dram_parameter("out", [S, D], BF16, isOutput=True)

    from contextlib import ExitStack

    with tile.TileContext(nc) as tc, ExitStack() as ctx:
        wpool = ctx.enter_context(tc.tile_pool(name="wpool", bufs=1))
        xpool = ctx.enter_context(tc.tile_pool(name="xpool", bufs=2))
        bigpool = ctx.enter_context(tc.tile_pool(name="bigpool", bufs=1))
        epool = ctx.enter_context(tc.tile_pool(name="epool", bufs=34))
        spool = ctx.enter_context(tc.tile_pool(name="spool", bufs=6))
        pp_acc = ctx.enter_context(tc.tile_pool(name="pp_acc", bufs=2, space="PSUM"))
        pp_sc = ctx.enter_context(tc.tile_pool(name="pp_sc", bufs=2, space="PSUM"))
        pp_ctx = ctx.enter_context(tc.tile_pool(name="pp_ctx", bufs=1, space="PSUM"))

        # ---- constants into SBUF ----
        wq_s = wpool.tile([P, KC2, KC, 2, P], F8, tag="wq")
        wk_s = wpool.tile([P, KC2, KC, 2, P], F8, tag="wk")
        wv_s = wpool.tile([P, KC, 2, 512], F8, tag="wv")
        wo_s = wpool.tile([P, KC2, 2, D], F8, tag="wo")
        bq_s = wpool.tile([P, KC2], F32, tag="bq")
        bk_s = wpool.tile([P, KC2], F32, tag="bk")
        pad_s = wpool.tile([P, S // P], F32, tag="pad")
        tri_s = wpool.tile([P, QW], BF16, tag="tri")
        eye_s = wpool.tile([P, P], BF16, tag="eye")
        # touch Exp once at t=0 so the ~1.3us ACT table load happens inside
        # the startup DMA shadow, not at the first real softmax
        warm_s = wpool.tile([P, 1], F32, tag="warm")
        nc.vector.memset(warm_s[:], 0.0)
        nc.scalar.activation(warm_s[:], warm_s[:],
                             mybir.ActivationFunctionType.Exp, scale=1.0)

        # ---- big persistent activations ----
        # q digits [pairdims, pair, {A,B}, q]; k digits [pairdims, pair, {K1,K2}, k]
        qd_s = bigpool.tile([P, KC2, 2, S], F8, tag="qd")
        kd_s = bigpool.tile([P, KC2, 2, S], F8, tag="kd")
        v_s = bigpool.tile([P, S // P, 8, HD + 1], BF16, tag="v")  # [k, ktile, h, hd|1]
        nc.vector.memset(v_s[:, :, :, HD:HD + 1], 1.0)
        # ctx digits [pairdims, pair, {c8,cr}, q]
        cd_s = bigpool.tile([P, KC2, 2, S], F8, tag="cd")

        def load_xt(st):
            xt_t = xpool.tile([P, KC, 2, QW], F8, tag="xt")
            nc.sync.dma_start(xt_t[:], xt[:, st, :, :, :])
            return xt_t

        def qkv_tiles(st, xt_t, parts="qkv", on_act=False, ms=None,
                      dig_dve=False):
            """Thunks projecting tokens [st*512, (st+1)*512): Q stripe st,
            K/V k-tiles 4*st..4*st+3, all fp8 DoubleRow 3-term. Each
            (matrix, m) splits into 3 PE emission units (~427ns each):
            main(4 DR), crossA(4 DR), crossB(4 DR)+psum->digit handoff."""
            ssl = slice(st * QW, (st + 1) * QW)
            thunks = []
            state = {}

            def qk_unit(w_s, b_s, dst, m, part, key):
                # part 0: main pairs; part 1: cross c=0..3; part 2: cross c=4..7
                # then qbf (DVE) + digit A (pool) + digit B (pool)
                if part == 0:
                    state[key] = pp_acc.tile([P, QW], F32, tag="acc",
                                             name="acc_ps")
                    ps = state[key]
                    for c in (0, 2, 4, 6):
                        nc.tensor.matmul(
                            ps[:], lhsT=w_s[:, m, c:c + 2, 1, :],
                            rhs=xt_t[:, c:c + 2, 0, :],
                            start=(c == 0), stop=False, perf_mode=DR)
                else:
                    ps = state[key]
                    for c in range(4 * (part - 1), 4 * part):
                        nc.tensor.matmul(
                            ps[:], lhsT=w_s[:, m, c, 0:2, :],
                            rhs=xt_t[:, c, 0:2, :],
                            start=False, stop=(c == KC - 1), perf_mode=DR)
                    if part == 2:
                        qbf = spool.tile([P, QW], BF16, tag="qbf",
                                         name="qbf", bufs=3)
                        nc.vector.tensor_scalar_add(qbf[:], ps[:],
                                                    b_s[:, m:m + 1])
                        del state[key]
                        # DVE for the startup digits (Pool's Q7 launches
                        # would gate the first scores), Pool afterwards
                        eng = nc.vector if dig_dve else nc.gpsimd
                        eng.tensor_scalar_mul(
                            dst[:, m, 0, ssl], qbf[:], 0.5)
                        eng.tensor_tensor(
                            dst[:, m, 1, ssl], qbf[:], dst[:, m, 0, ssl],
                            mybir.AluOpType.subtract)

            def v_unit(sub, part, key):
                subsl = slice(sub * P, (sub + 1) * P)
                if part == 0:
                    state[key] = pp_acc.tile([P, 8, HD], F32, tag="acc",
                                             name="acc_ps")
                    ps = state[key]
                    for c in (0, 2, 4, 6):
                        nc.tensor.matmul(
                            ps[:], lhsT=xt_t[:, c:c + 2, 0, subsl],
                            rhs=wv_s[:, c:c + 2, 1, :],
                            start=(c == 0), stop=False, perf_mode=DR)
                else:
                    ps = state[key]
                    for c in range(4 * (part - 1), 4 * part):
                        nc.tensor.matmul(
                            ps[:], lhsT=xt_t[:, c, 0:2, subsl],
                            rhs=wv_s[:, c, 0:2, :],
                            start=False, stop=(c == KC - 1), perf_mode=DR)
                    if part == 2:
                        nc.vector.tensor_copy(
                            out=v_s[:, st * 4 + sub, :, 0:HD], in_=ps[:])
                        del state[key]

            plan = []
            if "q" in parts:
                plan.append(("q", wq_s, bq_s, qd_s))
            if "k" in parts:
                plan.append(("k", wk_s, bk_s, kd_s))
            for pn, w_s, b_s, dst in plan:
                for m in (range(KC2) if ms is None else ms):
                    for part in range(3):
                        tag = (st, pn, m) if part == 2 else None
                        thunks.append(
                            (427, lambda w_s=w_s, b_s=b_s, dst=dst, m=m,
                             part=part, key=(pn, m):
                             qk_unit(w_s, b_s, dst, m, part, key), tag))
            if "v" in parts:
                for sub in range(4):
                    for part in range(3):
                        tag = (st, "v", sub) if part == 2 else None
                        thunks.append((427, lambda sub=sub, part=part,
                                       key=("v", sub): v_unit(sub, part, key),
                                       tag))
            return thunks

        def qkv_stage(st, xt_t, parts="qkv", ms=None, dig_dve=False):
            for _, t, tag in qkv_tiles(st, xt_t, parts, ms=ms,
                                       dig_dve=dig_dve):
                t()
                if tag:
                    done_tags.add(tag)

        from collections import deque
        fillers = deque()   # (ns, thunk, tag-or-None)
        done_tags = set()

        def pop_filler():
            ns, t, tag = fillers.popleft()
            t()
            if tag:
                done_tags.add(tag)
            return ns

        def ensure(tag):
            while tag not in done_tags and fillers:
                pop_filler()

        def drain():
            while fillers:
                pop_filler()

        attn_state = {}

        def attn_ph1(stripe, pr, j):
            """Scores/exp/mask for one (pair, k-tile); e kept in SBUF.
            One DR matmul per head: slots (K1,A)+(K2,B) ~= q16.k16/2."""
            es = attn_state[(stripe, pr)]
            m = j
            ksl = slice(m * P, (m + 1) * P)
            diag = m >= 4 * stripe
            # within a diagonal k-tile of shift t = m-4s, queries below
            # t*128 are entirely masked -- compute only the valid sub-range
            off = (m - 4 * stripe) * P if diag else 0
            w = QW - off
            qsub = slice(stripe * QW + off, (stripe + 1) * QW)
            sc = pp_sc.tile([P, 2 * QW], F32, tag="sc")
            for q_i in range(2):
                lo = q_i * HD
                nc.tensor.matmul(
                    sc[:, q_i * QW + off:(q_i + 1) * QW],
                    lhsT=kd_s[lo:lo + HD, pr, 0:2, ksl],
                    rhs=qd_s[lo:lo + HD, pr, 0:2, qsub],
                    start=True, stop=True, perf_mode=DR,
                    tile_position=(lo, 0))
            e = epool.tile([P, 2 * QW], BF16, tag="e")
            es.append(e)
            if off >= 256:
                # narrow diagonal exps: two short instructions beat one
                # full-width one once off >= 256 (ACT is the critical engine)
                for q_i in range(2):
                    esl = slice(q_i * QW + off, (q_i + 1) * QW)
                    nc.scalar.activation(e[:, esl], sc[:, esl],
                                         mybir.ActivationFunctionType.Exp,
                                         bias=pad_s[:, m:m + 1],
                                         scale=ESCALE)
            else:
                # full-width exp: any masked query columns hold garbage
                # (stale PSUM) but are never read downstream
                nc.scalar.activation(e[:], sc[:],
                                     mybir.ActivationFunctionType.Exp,
                                     bias=pad_s[:, m:m + 1],
                                     scale=ESCALE)
            if diag:
                # only the 128-wide diagonal query block needs the triangle;
                # beyond it tri is all-ones (no-op)
                for q_i in range(2):
                    esl = slice(q_i * QW + off, q_i * QW + off + P)
                    nc.vector.tensor_tensor(
                        e[:, esl], e[:, esl], tri_s[:, 0:P],
                        mybir.AluOpType.mult)

        lastcbf = {}

        def attn_ph2_parts(stripe, pr, last=False):
            """Post-phase-1 work for a pair. Returns {"region": fn,
            "units": [...]}. Normal pairs: units include the 8 ctx region
            bursts and end with the fp8 digit split of the transposed ctx.
            last=True: regions are called eagerly by the caller inside the
            ph1 j-loop, norms read PSUM directly (skip craw), and the digit
            split is skipped -- the tail oproj closes consume the bf16
            transpose staging buffer instead."""
            qsl = slice(stripe * QW, (stripe + 1) * QW)
            es = attn_state[(stripe, pr)]
            nkt = 4 * stripe + 4
            st = {}

            def region_span(q_i, qb, j0, j1, first, final):
                """ctx accumulation for js [j0, j1] of region (q_i, qb)"""
                if first and (q_i, qb) == (0, 0):
                    st["ctx_ps"] = pp_ctx.tile([P, 2, QW], F32, tag="ctx",
                                               name="ctx_ps")
                ctx_ps = st["ctx_ps"]
                h = 2 * pr + q_i
                js = list(range(j0, j1 + 1))
                for i, j in enumerate(js):
                    nc.tensor.matmul(
                        ctx_ps[:, q_i, qb * (HD + 1):(qb + 1) * (HD + 1)],
                        lhsT=es[j][:, q_i * QW + qb * P:q_i * QW + (qb + 1) * P],
                        rhs=v_s[:, j, h, :],
                        start=(first and i == 0),
                        stop=(final and i == len(js) - 1),
                        skip_group_check=True)

            def region(q_i, qb):
                region_span(q_i, qb, 0, 4 * stripe + qb, True, True)

            def recip():
                st["rden"] = spool.tile([P, 2, 4], F32, tag="rden",
                                        name="rden", bufs=2)
                nc.vector.reciprocal(
                    st["rden"][:],
                    st["ctx_ps"][:, :, HD:4 * (HD + 1):HD + 1])

            def recip_qb(qb):
                if "rden" not in st:
                    st["rden"] = spool.tile([P, 2, 4], F32, tag="rden",
                                            name="rden", bufs=2)
                nc.vector.reciprocal(
                    st["rden"][:, :, qb:qb + 1],
                    st["ctx_ps"][:, :, qb * (HD + 1) + HD:
                                 qb * (HD + 1) + HD + 1])

            def copyraw():
                # one bf16 copy frees the 2-bank ctx slot immediately; the
                # per-region normalizes then read SBUF at 2x DVE rate
                st["craw"] = spool.tile([P, 2, QW], BF16, tag="craw",
                                        name="craw", bufs=2)
                nc.vector.tensor_copy(out=st["craw"][:], in_=st["ctx_ps"][:])
                del st["ctx_ps"]
                attn_state[(stripe, pr)] = []  # release e tiles

            def norm(q_i, qb):
                if last:
                    src = st["ctx_ps"][:, q_i, qb * (HD + 1):qb * (HD + 1) + HD]
                else:
                    src = st["craw"][:, q_i, qb * (HD + 1):qb * (HD + 1) + HD]
                nc.vector.tensor_scalar_mul(
                    st["ctxc"][:, (q_i * 4 + qb) * HD:(q_i * 4 + qb + 1) * HD],
                    src,
                    st["rden"][:, q_i, qb:qb + 1])

            def mkctxc():
                st["ctxc"] = spool.tile([P, QW], BF16, tag="ctxc",
                                        name="ctxc", bufs=2)

            def transpose():
                st["pt"] = pp_ctx.tile([P, 2, QW], F32, tag="ctx", name="pt")
                for q_i in range(2):
                    for qb in range(4):
                        nc.tensor.matmul(
                            st["pt"][q_i * HD:(q_i + 1) * HD, 0,
                                     qb * P:(qb + 1) * P],
                            lhsT=st["ctxc"][:, (q_i * 4 + qb) * HD:
                                            (q_i * 4 + qb + 1) * HD],
                            rhs=eye_s[:],
                            start=True, stop=True,
                            skip_group_check=True)

            def ptcopy():
                # stage psum -> bf16, then split to fp8 digits on Pool
                st["cbf"] = spool.tile([P, QW], BF16, tag="cbf",
                                       name="cbf", bufs=2)
                nc.vector.tensor_copy(out=st["cbf"][:], in_=st["pt"][:, 0, :])
                del st["pt"]
                if last:
                    lastcbf[0] = st["cbf"]
                    del attn_state[(stripe, pr)]

            def cdig():
                nc.gpsimd.tensor_scalar_mul(cd_s[:, pr, 0, qsl],
                                            st["cbf"][:], 1.0)
                nc.gpsimd.tensor_tensor(cd_s[:, pr, 1, qsl], st["cbf"][:],
                                        cd_s[:, pr, 0, qsl],
                                        mybir.AluOpType.subtract)
                del attn_state[(stripe, pr)]

            if last:
                # per-qb finish: reciprocal + both heads' normalizes as soon
                # as that qb's denominator lands, inside the exp shadow
                def early_finish(qb):
                    if qb == 0:
                        mkctxc()
                    recip_qb(qb)
                    norm(0, qb)
                    norm(1, qb)

                units = [(0, lambda: st.pop("ctx_ps")),
                         (427, transpose), (0, ptcopy)]
                base = 4 * stripe
                # The backend invalidates a whole PSUM bank on start=True, so
                # a staggered region's start would race the (lagging) DVE
                # reads of an earlier qb's normalize. Instead: memset the ctx
                # psum once up front and run EVERY region accumulate-only
                # (start=False) -- no starts, no bank wipes, reads are safe.
                def zero_ps():
                    st["ctx_ps"] = pp_ctx.tile([P, 2, QW], F32, tag="ctx",
                                               name="ctx_ps")
                    nc.vector.memset(st["ctx_ps"][:], 0.0)

                return {"region": region, "early_finish": early_finish,
                        "zero": zero_ps,
                        "early": (lambda q_i, qb: region_span(
                            q_i, qb, 0, base + qb - 2, False, True)),
                        "tail": (lambda q_i, qb: region_span(
                            q_i, qb, base + qb - 1, base + qb, False, True)),
                        "units": units}
            units = [(27 * (4 * stripe + qb + 1),
                      lambda q_i=q_i, qb=qb: region(q_i, qb))
                     for q_i in range(2) for qb in range(4)]
            units += [(0, recip), (0, copyraw), (0, mkctxc)]
            units += [(0, lambda q_i=q_i, qb=qb: norm(q_i, qb))
                      for q_i in range(2) for qb in range(4)]
            units += [(427, transpose), (0, ptcopy), (0, cdig)]
            return {"region": region, "units": units}

        def attn_ph2_units(stripe, pr):
            return attn_ph2_parts(stripe, pr)["units"]

        pending = deque()  # ph2 units of the previously finished pair
        carry = [0]        # un-met PE deficit banked across k-tiles

        def attn_pair(stripe, pr, budget_ns=600, last=False, pre_pending=None):
            """Phase 1 j-loop for one pair, interleaving ph2 units of the
            previous pair (and fillers) between k-tiles. The exp costs
            ~1038ns/k-tile vs ~213ns of DR scores, so ~800ns of PE filler
            per k-tile keeps the PE dense; unmet remainder banks forward.
            last=True runs this pair's ctx regions eagerly inside the j-loop
            and its normalize/transpose chain immediately after, shortening
            the post-exp tail."""
            nkt = 4 * stripe + 4
            attn_state[(stripe, pr)] = []
            parts = attn_ph2_parts(stripe, pr, last=True) if last else None
            # deferred prerequisite of this pair's PENDING pops (not of ph1):
            # run it lazily so the first ph1/exp is already emitted before
            # the forced PE burst
            pre = [pre_pending] if pre_pending else []
            for j in range(nkt):
                attn_ph1(stripe, pr, j)
                if last and j >= 4 * stripe:
                    # staggered eager ph2: the bulk of each region (whose e
                    # tiles are all >=1 round old, so the in-order PE never
                    # stalls on a fresh exp) runs 2 rounds early; only a
                    # 2-matmul tail + recip/normalize trail each exp
                    d = j - 4 * stripe
                    if d == 0:
                        # the previous pair's ph2 must be fully emitted
                        # before this pair claims the single ctx PSUM slot
                        while pending:
                            pending.popleft()[1]()
                        parts["zero"]()
                        for qb in (0, 1):
                            parts["early"](0, qb)
                            parts["early"](1, qb)
                    else:
                        parts["tail"](0, d - 1)
                        parts["tail"](1, d - 1)
                        parts["early_finish"](d - 1)
                        if d + 1 <= 3:
                            parts["early"](0, d + 1)
                            parts["early"](1, d + 1)
                budget = carry[0] + budget_ns
                while budget > 100:
                    if pending:
                        if pre:
                            ensure(pre.pop())
                        ns, t = pending.popleft()
                        t()
                    elif fillers:
                        ns = pop_filler()
                    else:
                        break
                    budget -= max(ns, 50)
                carry[0] = min(max(budget, 0), B_CARRY)
            if last:
                parts["tail"](0, 3)
                parts["tail"](1, 3)
                parts["early_finish"](3)
                for _, t in parts["units"]:
                    t()
            else:
                pending.extend(attn_ph2_units(stripe, pr))

        def oproj_tiles(t8, alt=False, quarters=False):
            """Thunks for one 128-token output block: per 512-dout half,
            6 DR instrs (2 main chunk-pairs + 4 cross), ~640ns, or three
            ~213ns thirds (quarters=True)."""
            osl = slice(t8 * P, (t8 + 1) * P)
            state = {}

            def mm_main(ps, cs, start):
                for c in cs:
                    nc.tensor.matmul(ps[:], lhsT=cd_s[:, c:c + 2, 0, osl],
                                     rhs=wo_s[:, c:c + 2, 1,
                                              state["dsl"]],
                                     start=(start and c == cs[0]), stop=False,
                                     perf_mode=DR)

            def mm_cross(ps, kcs, stop):
                for kc in kcs:
                    nc.tensor.matmul(ps[:], lhsT=cd_s[:, kc, 0:2, osl],
                                     rhs=wo_s[:, kc, 0:2, state["dsl"]],
                                     start=False, stop=(stop and kc == kcs[-1]),
                                     perf_mode=DR)

            def fin(dt, ps):
                # both 512-halves stage into one tile; a single [128, 1024]
                # DMA per token block halves HWDGE/semaphore traffic
                if dt == 0:
                    state["ob"] = spool.tile([P, 2, QW], BF16, tag="outsb",
                                             name="ob", bufs=3)
                ob = state["ob"]
                nc.vector.tensor_copy(out=ob[:, dt, :], in_=ps[:])
                if dt == 1:
                    (nc.scalar if alt else nc.sync).dma_start(
                        out[osl, :], ob[:])
                    del state["ob"]

            def whole(dt):
                state["dsl"] = slice(dt * QW, (dt + 1) * QW)
                if alt and dt == 1:
                    ps = pp_sc.tile([P, 2 * QW], F32, tag="sc",
                                    name="oproj_ps")[:, 0:QW]
                else:
                    ps = pp_acc.tile([P, QW], F32, tag="acc", name="oproj_ps")
                mm_main(ps, [0, 2], True)
                mm_cross(ps, range(KC2), True)
                fin(dt, ps)

            def qopen(dt):
                state["dsl"] = slice(dt * QW, (dt + 1) * QW)
                state[dt] = pp_acc.tile([P, QW], F32, tag="acc",
                                        name="oproj_ps")
                mm_main(state[dt], [0, 2], True)
                mm_cross(state[dt], [0, 1], False)

            def qclose(dt):
                state["dsl"] = slice(dt * QW, (dt + 1) * QW)
                ps = state.pop(dt)
                mm_cross(ps, [2, 3], True)
                fin(dt, ps)

            if quarters:
                return [lambda dt=dt, f=f: f(dt)
                        for dt in range(2) for f in (qopen, qclose)]
            return [lambda dt=dt: whole(dt) for dt in range(2)]

        def oproj(t8, alt=False):
            for t in oproj_tiles(t8, alt):
                t()

        heldpart = {}

        def oproj_openA(t8, dt):
            """Pairs 0/1 products of a final oproj tile -- legal as soon as
            cdig(3,1) has popped, i.e. inside pair (3,2)'s window where the
            PE otherwise starves. Staged to SBUF bf16; the close replays
            pair 2 + the bf16 last-pair products and adds this partial."""
            ps = pp_acc.tile([P, QW], F32, tag="acc", name="oproj_ps")
            dsl = slice(dt * QW, (dt + 1) * QW)
            osl = slice(t8 * P, (t8 + 1) * P)
            nc.tensor.matmul(ps[:], lhsT=cd_s[:, 0:2, 0, osl],
                             rhs=wo_s[:, 0:2, 1, dsl],
                             start=True, stop=False, perf_mode=DR)
            for kc in range(2):
                nc.tensor.matmul(ps[:], lhsT=cd_s[:, kc, 0:2, osl],
                                 rhs=wo_s[:, kc, 0:2, dsl],
                                 start=False, stop=(kc == 1), perf_mode=DR)
            part = spool.tile([P, QW], BF16, tag="opart",
                              name=f"opart{t8}_{dt}", bufs=8)
            nc.vector.tensor_copy(out=part[:], in_=ps[:])
            heldpart[(t8, dt)] = part

        heldob = {}

        heldps = {}

        def oproj_close(t8, dt):
            part = heldpart.pop((t8, dt))
            cbf = lastcbf[0]
            tloc = slice((t8 - 12) * P, (t8 - 11) * P)
            ps = pp_acc.tile([P, QW], F32, tag="acc", name="oproj_ps")
            dsl = slice(dt * QW, (dt + 1) * QW)
            osl = slice(t8 * P, (t8 + 1) * P)
            nc.tensor.matmul(ps[:], lhsT=cd_s[:, 2, 0:2, osl],
                             rhs=wo_s[:, 2, 0:2, dsl],
                             start=True, stop=False, perf_mode=DR)
            nc.tensor.matmul(ps[:], lhsT=cd_s[:, 2, 0, osl],
                             rhs=wo_s[:, 2, 1, dsl],
                             start=False, stop=False)
            nc.tensor.matmul(ps[:], lhsT=cbf[:, tloc],
                             rhs=wo_s[:, 3, 1, dsl],
                             start=False, stop=False)
            nc.tensor.matmul(ps[:], lhsT=cbf[:, tloc],
                             rhs=wo_s[:, 3, 0, dsl],
                             start=False, stop=True)
            if dt == 0:
                heldob[t8] = spool.tile([P, 2, QW], BF16, tag="outsb",
                                        name="ob", bufs=3)
            ob = heldob[t8]
            nc.vector.tensor_tensor(ob[:, dt, :], ps[:], part[:],
                                    mybir.AluOpType.add)
            # per-half DMA: each half ships as soon as its add lands
            (nc.sync if t8 % 2 else nc.scalar).dma_start(out[osl, dsl],
                                                         ob[:, dt, :])
            if dt == 1:
                del heldob[t8]

        # ---- schedule ----
        # Startup DMAs: the minimal set for pair (0,0) first (xt0, m0 of
        # Wq/Wk, biases, pad, tri), then attention starts while the rest of
        # the weights stream in and the remaining stage work rides fillers.
        xt0 = xpool.tile([P, KC, 2, QW], F8, tag="xt")
        nc.scalar.dma_start(xt0[:, 0:2, :, :], xt[:, 0, 0:2, :, :])
        nc.sync.dma_start(xt0[:, 4:6, :, :], xt[:, 0, 4:6, :, :])
        # PE warmup on a zeroed tile: keeps the PE continuously busy through
        # the startup DMA shadow so the first real matmuls run at full clock
        # (the cost model's p-state ramp needs ~3us of uninterrupted work)
        wmm = wpool.tile([P, QW], BF16, tag="wmm")
        nc.gpsimd.memset(wmm[:], 0.0)
        wps = pp_sc.tile([P, 2 * QW], F32, tag="sc", name="warm_ps")
        for i in range(WARM_N):
            nc.tensor.matmul(wps[:, 0:QW], lhsT=wmm[:, 0:P], rhs=wmm[:],
                             start=(i == 0), stop=(i == WARM_N - 1))
        nc.scalar.dma_start(xt0[:, 2:4, :, :], xt[:, 0, 2:4, :, :])
        nc.sync.dma_start(wk_s[:, 0, :, :, :], wk[:, 0, :, :, :])
        nc.scalar.dma_start(xt0[:, 6:8, :, :], xt[:, 0, 6:8, :, :])
        nc.sync.dma_start(wq_s[:, 0, :, :, :], wq[:, 0, :, :, :])
        nc.scalar.dma_start(bk_s[:], bkp[:])
        nc.sync.dma_start(bq_s[:], bqp[:])
        nc.scalar.dma_start(tri_s[:], tri[:])
        nc.sync.dma_start(pad_s[:], pad[:])
        # q digits on DVE, k digits on Pool: the two first-pair digit chains
        # run on different engines in parallel
        qkv_stage(0, xt0, parts="q", ms=[0], dig_dve=True)
        qkv_stage(0, xt0, parts="k", ms=[0])
        nc.sync.dma_start(wv_s[:, 0:4, :, :], wv[:, 0:4, :, :])
        nc.scalar.dma_start(wv_s[:, 4:8, :, :], wv[:, 4:8, :, :])
        xt1 = load_xt(1)
        nc.sync.dma_start(wq_s[:, 1, :, :, :], wq[:, 1, :, :, :])
        nc.scalar.dma_start(wk_s[:, 1, :, :, :], wk[:, 1, :, :, :])
        nc.sync.dma_start(wq_s[:, 2, :, :, :], wq[:, 2, :, :, :])
        nc.scalar.dma_start(wk_s[:, 2, :, :, :], wk[:, 2, :, :, :])
        nc.sync.dma_start(wq_s[:, 3, :, :, :], wq[:, 3, :, :, :])
        nc.scalar.dma_start(wk_s[:, 3, :, :, :], wk[:, 3, :, :, :])
        nc.scalar.dma_start(eye_s[:], eye[:])

        # stripe 0/1 pair interleave: stripe 0 alone supplies too little exp
        # work to keep ACT busy through the projection-heavy opening, so
        # stripe-1 pairs (2x the exp volume) run in between
        fillers.extend(qkv_tiles(1, xt1, parts="q", ms=[0]))
        fillers.extend(qkv_tiles(1, xt1, parts="k", ms=[0]))
        fillers.extend(qkv_tiles(0, xt0, parts="v"))
        fillers.extend(qkv_tiles(0, xt0, parts="q", ms=[1]))
        fillers.extend(qkv_tiles(0, xt0, parts="k", ms=[1]))
        fillers.extend(qkv_tiles(1, xt1, parts="v"))
        for m in (1, 2, 3):
            sts = (1,) if m == 1 else (0, 1)
            for stq in sts:
                xtt = xt0 if stq == 0 else xt1
                fillers.extend(qkv_tiles(stq, xtt, parts="q", ms=[m]))
                fillers.extend(qkv_tiles(stq, xtt, parts="k", ms=[m]))
        nc.sync.dma_start(wo_s[:], wo[:])
        attn_pair(0, 0, budget_ns=B_P00)
        ensure((1, "q", 0))
        ensure((1, "k", 0))
        ensure((0, "v", 3))
        attn_pair(1, 0, budget_ns=B_EARLY)
        ensure((0, "q", 1))
        ensure((0, "k", 1))
        ensure((1, "v", 3))
        attn_pair(0, 1, budget_ns=B_EARLY)
        ensure((1, "q", 1))
        ensure((1, "k", 1))
        attn_pair(1, 1, budget_ns=B_EARLY)
        ensure((0, "q", 2))
        ensure((0, "k", 2))
        attn_pair(0, 2, budget_ns=B_EARLY)
        ensure((1, "q", 2))
        ensure((1, "k", 2))
        attn_pair(1, 2, budget_ns=B_EARLY)
        ensure((0, "q", 3))
        ensure((0, "k", 3))
        attn_pair(0, 3, budget_ns=B_EARLY)
        ensure((1, "q", 3))
        ensure((1, "k", 3))
        xt2 = load_xt(2)

        def extend_stage_qk(st, xt_t):
            for m in range(KC2):
                fillers.extend(qkv_tiles(st, xt_t, parts="q", ms=[m]))
                fillers.extend(qkv_tiles(st, xt_t, parts="k", ms=[m]))

        extend_stage_qk(2, xt2)
        fillers.extend(qkv_tiles(2, xt2, parts="v"))
        attn_pair(1, 3, budget_ns=B_EARLY)
        xt3 = load_xt(3)
        extend_stage_qk(3, xt3)
        for pr in range(4):
            ensure((2, "q", pr))
            ensure((2, "k", pr))
            if pr == 1:
                ensure((2, "v", 3))
            attn_pair(2, pr, budget_ns=B_MID)
        # V(3) is safe here -- first needed by the ctx regions of pair (3,0),
        # which only run during ph1(3,1). oproj of stripe-2 blocks becomes
        # legal once pair (2,3)'s pending units pop at the start of (3,0).
        fillers.extend(qkv_tiles(3, xt3, parts="v"))
        for t8 in range(0, 11):
            fillers.extend((213, t, None)
                           for t in oproj_tiles(t8, quarters=True))
        for pr in range(2):
            ensure((3, "q", pr))
            ensure((3, "k", pr))
            if pr == 1:
                ensure((3, "v", 3))
            attn_pair(3, pr, budget_ns=B_S3)
        # openA (pairs 0/1 products) of the final oproj tiles becomes legal
        # once cdig(3,1) pops -- feed it to pair (3,2)'s otherwise-starved
        # PE, together with the remaining stripe-2 oproj tile
        ensure((3, "q", 2))
        ensure((3, "k", 2))
        fillers.extend((213, t, None)
                       for t in oproj_tiles(11, quarters=True))
        for t8 in range(12, 16):
            fillers.append((320, (lambda t8=t8: oproj_openA(t8, 0)), None))
            fillers.append((320, (lambda t8=t8: oproj_openA(t8, 1)), None))
        attn_pair(3, 2, budget_ns=B_S3)
        ensure((3, "q", 3))
        ensure((3, "k", 3))
        attn_pair(3, 3, budget_ns=B_LAST, last=True)
        while pending:
            pending.popleft()[1]()
        drain()
        for t8 in range(12, 16):
            oproj_close(t8, 0)
            oproj_close(t8, 1)

    nc.compile()
    return nc


def _dig(a):
    """two-digit e4m3 split along a new axis: returns np [..., 2] fp8"""
    hi = a.astype(NPE4)
    lo = (a - hi.astype(np.float32)).astype(NPE4)
    return hi, lo


def _core_inputs(c, x, padding_mask, Wq, bq, Wk, bk, Wv, bv, Wo, bo):
    b, hh = c // 2, c % 2
    hsl = slice(hh * 512, (hh + 1) * 512)

    xb = np.ascontiguousarray(
        x[b].T.reshape(KC, P, S).transpose(1, 0, 2)).astype(np.float32)
    x8, xr = _dig(xb)
    # [P, KC, 2, S] -> stripe-major [P, NS, KC, 2, QW]
    xt = np.stack([x8, xr], axis=2).reshape(P, KC, 2, NS, QW)
    xt = np.ascontiguousarray(xt.transpose(0, 3, 1, 2, 4))

    def wl(Wh):  # [512 out, 1024 in] -> m-major [P, KC2, KC, 2, 128] {Wr,W8}
        w = np.ascontiguousarray(
            Wh.T.reshape(KC, P, 512).transpose(1, 0, 2)).astype(np.float32)
        w8, wr = _dig(WS * w)
        st = np.stack([wr, w8], axis=2)          # [P, KC, 2, 512]
        st = st.reshape(P, KC, 2, KC2, P).transpose(0, 3, 1, 2, 4)
        return np.ascontiguousarray(st)

    def wvl(Wh):  # [512 out, 1024 in] -> chunk-major [P, KC, 2, 512] {Wr,W8}
        w = np.ascontiguousarray(
            Wh.T.reshape(KC, P, 512).transpose(1, 0, 2)).astype(np.float32)
        w8, wr = _dig(WS * w)
        return np.ascontiguousarray(np.stack([wr, w8], axis=2))

    wob = np.ascontiguousarray(
        Wo[:, hsl].T.reshape(KC2, P, D).transpose(1, 0, 2)).astype(np.float32)
    wo8, wor = _dig(WS * wob)
    wol = np.ascontiguousarray(np.stack([wor, wo8], axis=2))

    bqp = np.ascontiguousarray(
        WS * bq[hsl].reshape(KC2, P).T).astype(np.float32)
    bkp = np.ascontiguousarray(
        WS * bk[hsl].reshape(KC2, P).T).astype(np.float32)

    padb = np.where(padding_mask[b].reshape(S // P, P).T, 0.0,
                    NEG).astype(np.float32)
    padb = np.ascontiguousarray(padb)

    kk = np.arange(P)[:, None]
    uu = np.arange(QW)[None, :]
    trib = np.ascontiguousarray((kk <= uu).astype(NPBF16))

    return {"xt": xt, "wq": wl(Wq[hsl]), "wk": wl(Wk[hsl]), "wv": wvl(Wv[hsl]),
            "wo": wol, "bqp": bqp, "bkp": bkp, "pad": padb, "tri": trib,
            "eye": np.eye(P, dtype=NPBF16)}


_NC_CACHE = {}


def kernel(x, padding_mask, Wq, bq, Wk, bk, Wv, bv, Wo, bo):
    x = np.asarray(x, np.float32)
    padding_mask = np.asarray(padding_mask, bool)
    args = [np.asarray(a, np.float32) for a in (Wq, bq, Wk, bk, Wv, bv, Wo, bo)]

    if "nc" not in _NC_CACHE:
        _NC_CACHE["nc"] = _build()
    nc = _NC_CACHE["nc"]

    in_maps = [_core_inputs(c, x, padding_mask, *args) for c in range(8)]

    trace = bool(int(os.environ.get("KERNEL_TRACE", "0")))
    try:
        res = run_bass_kernel_spmd(nc, in_maps, core_ids=list(range(8)), trace=trace)
    except ModuleNotFoundError:
        res = run_bass_kernel_spmd(nc, in_maps, core_ids=list(range(8)))
    if trace and res.exec_time_ns is not None:
        print(f"HW exec time: {res.exec_time_ns} ns")
        _NC_CACHE["exec_time_ns"] = res.exec_time_ns

    Wo_, bv_, bo_ = args[6], args[5], args[7]
    btot = (bo_ + Wo_ @ bv_).astype(np.float32)
    descale = 1.0 / (WS * WS)
    full = np.empty((B, S, D), np.float32)
    for b in range(B):
        full[b] = ((res.results[2 * b]["out"].astype(np.float32)
                    + res.results[2 * b + 1]["out"].astype(np.float32))
                   * descale + btot)
    return full


if __name__ == "__main__":
    rng = np.random.default_rng(0)
    x = rng.standard_normal((B, S, D), dtype=np.float32)
    lengths = rng.integers(S // 2, S + 1, size=(B,))
    pm = np.arange(S)[None, :] < lengths[:, None]
    std = 0.02
    ws = {n: (rng.standard_normal((D, D), dtype=np.float32) * std)
          for n in ("Wq", "Wk", "Wv", "Wo")}
    z = np.zeros((D,), np.float32)
    out = kernel(x, pm, ws["Wq"], z, ws["Wk"], z, ws["Wv"], z, ws["Wo"], z)
    print(out.shape, out.dtype, np.abs(out).mean())


# revision 6
# speedup vs baseline: 1.0223x; 1.0039x over previous
"""Causal MHA (B=4, S=2048, D=1024, H=16) on 8 TRN2 cores, head-parallel,
fp8 DoubleRow edition.

Core c = (batch b=c//2, head-half hh=c%2). Same schedule skeleton as the
bf16 baseline, but every projection matmul runs fp8e4m3 DoubleRow:

- QKV/O projections: 3-term compensated digits (x = x8+xr, 16W = W8+Wr,
  dropping the xr*Wr term) -> bf16-level accuracy at 6/8 the bf16 PE cost.
  Digit pairs pack into DR slots: main instrs pair (W8_c, W8_c+1) x
  (x8_c, x8_c+1); cross instrs pair (Wr_c, W8_c) x (x8_c, xr_c).
- Scores: twin-sample split q16 = A+B (A = e4(q16/2), B = e4(q16-A)),
  k16 = K1+K2; one DR instr per head per k-tile computes K1.A + K2.B
  ~= q16.k16/2 with ~2.5% rms error (vs 3.6% single-digit), at HALF the
  bf16 score cost. exp scale absorbs the 2/(256*8) factor.
- ctx + transpose stay bf16 (e-quantization to fp8 would break the 2e-2
  gate). ctx values carry a 16x scale (v = x@(16Wv)); the oproj digits
  c8/cr quantize the 16x-scaled ctx (good e4m3 range), and the host
  divides partial outputs by 256.

fp8 digit production rides the idle GpSimd(Pool) engine + DVE slack.
"""

import os
import sys

sys.path.insert(0, "/opt/trn_rl_repo")

import numpy as np
import ml_dtypes

import concourse.bass as bass
import concourse.bacc as bacc
import concourse.tile as tile
from concourse import mybir
from concourse.bass_utils import run_bass_kernel_spmd

B, S, D, H = 4, 2048, 1024, 16
HD = D // H  # 64
P = 128
KC = D // P   # 8 contraction chunks for QKV projections
KC2 = 4       # contraction chunks for O projection (512 dims)
QW = 512      # query stripe width
NS = S // QW  # 4 stripes
NEG = -1e30
BF16 = mybir.dt.bfloat16
F32 = mybir.dt.float32
F8 = mybir.dt.float8e4
NPBF16 = ml_dtypes.bfloat16
NPE4 = ml_dtypes.float8_e4m3
WS = 16.0           # host weight scale
B_EARLY = 600   # filler budget/k-tile: stripe 0/1 interleave
B_MID = 700     # stripe 1 tail + stripe 2
B_S3 = 900      # stripe 3 pairs 0-2
B_LAST = 1300   # final pair
B_CARRY = 1500  # carry cap
WARM_N = 8      # PE warmup matmuls
B_P00 = 700     # first pair budget
ESCALE = 2.0 / (WS * WS * 8.0)   # exp scale: twin-slot 2x / (16*16 * sqrt(hd))
DR = mybir.MatmulPerfMode.DoubleRow


def _build():
    nc = bacc.Bacc()

    # x digits, stripe-major: [indim-part, stripe, chunk, {x8,xr}, tok]
    xt = nc.declare_dram_parameter("xt", [P, NS, KC, 2, QW], F8, isOutput=False)
    # Wq/Wk digits, m-major: [indim-part, m, chunk, {Wr,W8}, 128 outdim]
    wq = nc.declare_dram_parameter("wq", [P, KC2, KC, 2, P], F8, isOutput=False)
    wk = nc.declare_dram_parameter("wk", [P, KC2, KC, 2, P], F8, isOutput=False)
    # Wv digits, chunk-major: [indim-part, chunk, {Wr,W8}, 512 outdim]
    wv = nc.declare_dram_parameter("wv", [P, KC, 2, 512], F8, isOutput=False)
    # Wo digits: [dh-part, chunk(=pair), {Wor,Wo8}, dout]
    wo = nc.declare_dram_parameter("wo", [P, KC2, 2, D], F8, isOutput=False)
    bqp = nc.declare_dram_parameter("bqp", [P, KC2], F32, isOutput=False)
    bkp = nc.declare_dram_parameter("bkp", [P, KC2], F32, isOutput=False)
    pad = nc.declare_dram_parameter("pad", [P, S // P], F32, isOutput=False)
    tri = nc.declare_dram_parameter("tri", [P, QW], BF16, isOutput=False)
    eye = nc.declare_dram_parameter("eye", [P, P], BF16, isOutput=False)
    out = nc.declare_dram_parameter("out", [S, D], BF16, isOutput=True)

    from contextlib import ExitStack

    with tile.TileContext(nc) as tc, ExitStack() as ctx:
        wpool = ctx.enter_context(tc.tile_pool(name="wpool", bufs=1))
        xpool = ctx.enter_context(tc.tile_pool(name="xpool", bufs=2))
        bigpool = ctx.enter_context(tc.tile_pool(name="bigpool", bufs=1))
        epool = ctx.enter_context(tc.tile_pool(name="epool", bufs=34))
        spool = ctx.enter_context(tc.tile_pool(name="spool", bufs=6))
        pp_acc = ctx.enter_context(tc.tile_pool(name="pp_acc", bufs=2, space="PSUM"))
        pp_sc = ctx.enter_context(tc.tile_pool(name="pp_sc", bufs=2, space="PSUM"))
        pp_ctx = ctx.enter_context(tc.tile_pool(name="pp_ctx", bufs=1, space="PSUM"))

        # ---- constants into SBUF ----
        wq_s = wpool.tile([P, KC2, KC, 2, P], F8, tag="wq")
        wk_s = wpool.tile([P, KC2, KC, 2, P], F8, tag="wk")
        wv_s = wpool.tile([P, KC, 2, 512], F8, tag="wv")
        wo_s = wpool.tile([P, KC2, 2, D], F8, tag="wo")
        bq_s = wpool.tile([P, KC2], F32, tag="bq")
        bk_s = wpool.tile([P, KC2], F32, tag="bk")
        pad_s = wpool.tile([P, S // P], F32, tag="pad")
        tri_s = wpool.tile([P, QW], BF16, tag="tri")
        eye_s = wpool.tile([P, P], BF16, tag="eye")
        # touch Exp once at t=0 so the ~1.3us ACT table load happens inside
        # the startup DMA shadow, not at the first real softmax
        warm_s = wpool.tile([P, 1], F32, tag="warm")
        nc.vector.memset(warm_s[:], 0.0)
        nc.scalar.activation(warm_s[:], warm_s[:],
                             mybir.ActivationFunctionType.Exp, scale=1.0)

        # ---- big persistent activations ----
        # q digits [pairdims, pair, {A,B}, q]; k digits [pairdims, pair, {K1,K2}, k]
        qd_s = bigpool.tile([P, KC2, 2, S], F8, tag="qd")
        kd_s = bigpool.tile([P, KC2, 2, S], F8, tag="kd")
        v_s = bigpool.tile([P, S // P, 8, HD + 1], BF16, tag="v")  # [k, ktile, h, hd|1]
        nc.vector.memset(v_s[:, :, :, HD:HD + 1], 1.0)
        # ctx digits [pairdims, pair, {c8,cr}, q]
        cd_s = bigpool.tile([P, KC2, 2, S], F8, tag="cd")

        def load_xt(st):
            xt_t = xpool.tile([P, KC, 2, QW], F8, tag="xt")
            nc.sync.dma_start(xt_t[:], xt[:, st, :, :, :])
            return xt_t

        def qkv_tiles(st, xt_t, parts="qkv", on_act=False, ms=None,
                      dig_dve=False):
            """Thunks projecting tokens [st*512, (st+1)*512): Q stripe st,
            K/V k-tiles 4*st..4*st+3, all fp8 DoubleRow 3-term. Each
            (matrix, m) splits into 3 PE emission units (~427ns each):
            main(4 DR), crossA(4 DR), crossB(4 DR)+psum->digit handoff."""
            ssl = slice(st * QW, (st + 1) * QW)
            thunks = []
            state = {}

            def qk_unit(w_s, b_s, dst, m, part, key):
                # part 0: main pairs; part 1: cross c=0..3; part 2: cross c=4..7
                # then qbf (DVE) + digit A (pool) + digit B (pool)
                if part == 0:
                    state[key] = pp_acc.tile([P, QW], F32, tag="acc",
                                             name="acc_ps")
                    ps = state[key]
                    for c in (0, 2, 4, 6):
                        nc.tensor.matmul(
                            ps[:], lhsT=w_s[:, m, c:c + 2, 1, :],
                            rhs=xt_t[:, c:c + 2, 0, :],
                            start=(c == 0), stop=False, perf_mode=DR)
                else:
                    ps = state[key]
                    for c in range(4 * (part - 1), 4 * part):
                        nc.tensor.matmul(
                            ps[:], lhsT=w_s[:, m, c, 0:2, :],
                            rhs=xt_t[:, c, 0:2, :],
                            start=False, stop=(c == KC - 1), perf_mode=DR)
                    if part == 2:
                        qbf = spool.tile([P, QW], BF16, tag="qbf",
                                         name="qbf", bufs=3)
                        nc.vector.tensor_scalar_add(qbf[:], ps[:],
                                                    b_s[:, m:m + 1])
                        del state[key]
                        # DVE for the startup digits (Pool's Q7 launches
                        # would gate the first scores), Pool afterwards
                        eng = nc.vector if dig_dve else nc.gpsimd
                        eng.tensor_scalar_mul(
                            dst[:, m, 0, ssl], qbf[:], 0.5)
                        eng.tensor_tensor(
                            dst[:, m, 1, ssl], qbf[:], dst[:, m, 0, ssl],
                            mybir.AluOpType.subtract)

            def v_unit(sub, part, key):
                subsl = slice(sub * P, (sub + 1) * P)
                if part == 0:
                    state[key] = pp_acc.tile([P, 8, HD], F32, tag="acc",
                                             name="acc_ps")
                    ps = state[key]
                    for c in (0, 2, 4, 6):
                        nc.tensor.matmul(
                            ps[:], lhsT=xt_t[:, c:c + 2, 0, subsl],
                            rhs=wv_s[:, c:c + 2, 1, :],
                            start=(c == 0), stop=False, perf_mode=DR)
                else:
                    ps = state[key]
                    for c in range(4 * (part - 1), 4 * part):
                        nc.tensor.matmul(
                            ps[:], lhsT=xt_t[:, c, 0:2, subsl],
                            rhs=wv_s[:, c, 0:2, :],
                            start=False, stop=(c == KC - 1), perf_mode=DR)
                    if part == 2:
                        nc.vector.tensor_copy(
                            out=v_s[:, st * 4 + sub, :, 0:HD], in_=ps[:])
                        del state[key]

            plan = []
            if "q" in parts:
                plan.append(("q", wq_s, bq_s, qd_s))
            if "k" in parts:
                plan.append(("k", wk_s, bk_s, kd_s))
            for pn, w_s, b_s, dst in plan:
                for m in (range(KC2) if ms is None else ms):
                    for part in range(3):
                        tag = (st, pn, m) if part == 2 else None
                        thunks.append(
                            (427, lambda w_s=w_s, b_s=b_s, dst=dst, m=m,
                             part=part, key=(pn, m):
                             qk_unit(w_s, b_s, dst, m, part, key), tag))
            if "v" in parts:
                for sub in range(4):
                    for part in range(3):
                        tag = (st, "v", sub) if part == 2 else None
                        thunks.append((427, lambda sub=sub, part=part,
                                       key=("v", sub): v_unit(sub, part, key),
                                       tag))
            return thunks

        def qkv_stage(st, xt_t, parts="qkv", ms=None, dig_dve=False):
            for _, t, tag in qkv_tiles(st, xt_t, parts, ms=ms,
                                       dig_dve=dig_dve):
                t()
                if tag:
                    done_tags.add(tag)

        from collections import deque
        fillers = deque()   # (ns, thunk, tag-or-None)
        done_tags = set()

        def pop_filler():
            ns, t, tag = fillers.popleft()
            t()
            if tag:
                done_tags.add(tag)
            return ns

        def ensure(tag):
            while tag not in done_tags and fillers:
                pop_filler()

        def drain():
            while fillers:
                pop_filler()

        attn_state = {}

        def attn_ph1(stripe, pr, j):
            """Scores/exp/mask for one (pair, k-tile); e kept in SBUF.
            One DR matmul per head: slots (K1,A)+(K2,B) ~= q16.k16/2."""
            es = attn_state[(stripe, pr)]
            m = j
            ksl = slice(m * P, (m + 1) * P)
            diag = m >= 4 * stripe
            # within a diagonal k-tile of shift t = m-4s, queries below
            # t*128 are entirely masked -- compute only the valid sub-range
            off = (m - 4 * stripe) * P if diag else 0
            w = QW - off
            qsub = slice(stripe * QW + off, (stripe + 1) * QW)
            sc = pp_sc.tile([P, 2 * QW], F32, tag="sc")
            for q_i in range(2):
                lo = q_i * HD
                nc.tensor.matmul(
                    sc[:, q_i * QW + off:(q_i + 1) * QW],
                    lhsT=kd_s[lo:lo + HD, pr, 0:2, ksl],
                    rhs=qd_s[lo:lo + HD, pr, 0:2, qsub],
                    start=True, stop=True, perf_mode=DR,
                    tile_position=(lo, 0))
            e = epool.tile([P, 2 * QW], BF16, tag="e")
            es.append(e)
            if off >= 384:
                # narrow diagonal exps: two short instructions beat one
                # full-width one once off >= 384 (ACT is the critical engine)
                for q_i in range(2):
                    esl = slice(q_i * QW + off, (q_i + 1) * QW)
                    nc.scalar.activation(e[:, esl], sc[:, esl],
                                         mybir.ActivationFunctionType.Exp,
                                         bias=pad_s[:, m:m + 1],
                                         scale=ESCALE)
            elif off in (128, 256):
                # one shifted instruction [off:1024]: skips head0's masked
                # prefix; the mid-span garbage is bounded and never read
                nc.scalar.activation(e[:, off:], sc[:, off:],
                                     mybir.ActivationFunctionType.Exp,
                                     bias=pad_s[:, m:m + 1],
                                     scale=ESCALE)
            else:
                # full-width exp: any masked query columns hold garbage
                # (stale PSUM) but are never read downstream
                nc.scalar.activation(e[:], sc[:],
                                     mybir.ActivationFunctionType.Exp,
                                     bias=pad_s[:, m:m + 1],
                                     scale=ESCALE)
            if diag:
                # only the 128-wide diagonal query block needs the triangle;
                # beyond it tri is all-ones (no-op)
                for q_i in range(2):
                    esl = slice(q_i * QW + off, q_i * QW + off + P)
                    nc.vector.tensor_tensor(
                        e[:, esl], e[:, esl], tri_s[:, 0:P],
                        mybir.AluOpType.mult)

        lastcbf = {}

        def attn_ph2_parts(stripe, pr, last=False):
            """Post-phase-1 work for a pair. Returns {"region": fn,
            "units": [...]}. Normal pairs: units include the 8 ctx region
            bursts and end with the fp8 digit split of the transposed ctx.
            last=True: regions are called eagerly by the caller inside the
            ph1 j-loop, norms read PSUM directly (skip craw), and the digit
            split is skipped -- the tail oproj closes consume the bf16
            transpose staging buffer instead."""
            qsl = slice(stripe * QW, (stripe + 1) * QW)
            es = attn_state[(stripe, pr)]
            nkt = 4 * stripe + 4
            st = {}

            def region_span(q_i, qb, j0, j1, first, final):
                """ctx accumulation for js [j0, j1] of region (q_i, qb)"""
                if first and (q_i, qb) == (0, 0):
                    st["ctx_ps"] = pp_ctx.tile([P, 2, QW], F32, tag="ctx",
                                               name="ctx_ps")
                ctx_ps = st["ctx_ps"]
                h = 2 * pr + q_i
                js = list(range(j0, j1 + 1))
                for i, j in enumerate(js):
                    nc.tensor.matmul(
                        ctx_ps[:, q_i, qb * (HD + 1):(qb + 1) * (HD + 1)],
                        lhsT=es[j][:, q_i * QW + qb * P:q_i * QW + (qb + 1) * P],
                        rhs=v_s[:, j, h, :],
                        start=(first and i == 0),
                        stop=(final and i == len(js) - 1),
                        skip_group_check=True)

            def region(q_i, qb):
                region_span(q_i, qb, 0, 4 * stripe + qb, True, True)

            def recip():
                st["rden"] = spool.tile([P, 2, 4], F32, tag="rden",
                                        name="rden", bufs=2)
                nc.vector.reciprocal(
                    st["rden"][:],
                    st["ctx_ps"][:, :, HD:4 * (HD + 1):HD + 1])

            def recip_qb(qb):
                if "rden" not in st:
                    st["rden"] = spool.tile([P, 2, 4], F32, tag="rden",
                                            name="rden", bufs=2)
                nc.vector.reciprocal(
                    st["rden"][:, :, qb:qb + 1],
                    st["ctx_ps"][:, :, qb * (HD + 1) + HD:
                                 qb * (HD + 1) + HD + 1])

            def copyraw():
                # one bf16 copy frees the 2-bank ctx slot immediately; the
                # per-region normalizes then read SBUF at 2x DVE rate
                st["craw"] = spool.tile([P, 2, QW], BF16, tag="craw",
                                        name="craw", bufs=2)
                nc.vector.tensor_copy(out=st["craw"][:], in_=st["ctx_ps"][:])
                del st["ctx_ps"]
                attn_state[(stripe, pr)] = []  # release e tiles

            def norm(q_i, qb):
                if last:
                    src = st["ctx_ps"][:, q_i, qb * (HD + 1):qb * (HD + 1) + HD]
                else:
                    src = st["craw"][:, q_i, qb * (HD + 1):qb * (HD + 1) + HD]
                nc.vector.tensor_scalar_mul(
                    st["ctxc"][:, (q_i * 4 + qb) * HD:(q_i * 4 + qb + 1) * HD],
                    src,
                    st["rden"][:, q_i, qb:qb + 1])

            def mkctxc():
                st["ctxc"] = spool.tile([P, QW], BF16, tag="ctxc",
                                        name="ctxc", bufs=2)

            def transpose():
                st["pt"] = pp_ctx.tile([P, 2, QW], F32, tag="ctx", name="pt")
                for q_i in range(2):
                    for qb in range(4):
                        nc.tensor.matmul(
                            st["pt"][q_i * HD:(q_i + 1) * HD, 0,
                                     qb * P:(qb + 1) * P],
                            lhsT=st["ctxc"][:, (q_i * 4 + qb) * HD:
                                            (q_i * 4 + qb + 1) * HD],
                            rhs=eye_s[:],
                            start=True, stop=True,
                            skip_group_check=True)

            def ptcopy():
                # stage psum -> bf16, then split to fp8 digits on Pool
                st["cbf"] = spool.tile([P, QW], BF16, tag="cbf",
                                       name="cbf", bufs=2)
                nc.vector.tensor_copy(out=st["cbf"][:], in_=st["pt"][:, 0, :])
                del st["pt"]
                if last:
                    lastcbf[0] = st["cbf"]
                    del attn_state[(stripe, pr)]

            def cdig():
                nc.gpsimd.tensor_scalar_mul(cd_s[:, pr, 0, qsl],
                                            st["cbf"][:], 1.0)
                nc.gpsimd.tensor_tensor(cd_s[:, pr, 1, qsl], st["cbf"][:],
                                        cd_s[:, pr, 0, qsl],
                                        mybir.AluOpType.subtract)
                del attn_state[(stripe, pr)]

            if last:
                # per-qb finish: reciprocal + both heads' normalizes as soon
                # as that qb's denominator lands, inside the exp shadow
                def early_finish(qb):
                    if qb == 0:
                        mkctxc()
                    recip_qb(qb)
                    norm(0, qb)
                    norm(1, qb)

                units = [(0, lambda: st.pop("ctx_ps")),
                         (427, transpose), (0, ptcopy)]
                base = 4 * stripe
                # The backend invalidates a whole PSUM bank on start=True, so
                # a staggered region's start would race the (lagging) DVE
                # reads of an earlier qb's normalize. Instead: memset the ctx
                # psum once up front and run EVERY region accumulate-only
                # (start=False) -- no starts, no bank wipes, reads are safe.
                def zero_ps():
                    st["ctx_ps"] = pp_ctx.tile([P, 2, QW], F32, tag="ctx",
                                               name="ctx_ps")
                    nc.vector.memset(st["ctx_ps"][:], 0.0)

                return {"region": region, "early_finish": early_finish,
                        "zero": zero_ps,
                        "early": (lambda q_i, qb: region_span(
                            q_i, qb, 0, base + qb - 2, False, True)),
                        "tail": (lambda q_i, qb: region_span(
                            q_i, qb, base + qb - 1, base + qb, False, True)),
                        "units": units}
            units = [(27 * (4 * stripe + qb + 1),
                      lambda q_i=q_i, qb=qb: region(q_i, qb))
                     for q_i in range(2) for qb in range(4)]
            units += [(0, recip), (0, copyraw), (0, mkctxc)]
            units += [(0, lambda q_i=q_i, qb=qb: norm(q_i, qb))
                      for q_i in range(2) for qb in range(4)]
            units += [(427, transpose), (0, ptcopy), (0, cdig)]
            return {"region": region, "units": units}

        def attn_ph2_units(stripe, pr):
            return attn_ph2_parts(stripe, pr)["units"]

        pending = deque()  # ph2 units of the previously finished pair
        carry = [0]        # un-met PE deficit banked across k-tiles

        def attn_pair(stripe, pr, budget_ns=600, last=False, pre_pending=None):
            """Phase 1 j-loop for one pair, interleaving ph2 units of the
            previous pair (and fillers) between k-tiles. The exp costs
            ~1038ns/k-tile vs ~213ns of DR scores, so ~800ns of PE filler
            per k-tile keeps the PE dense; unmet remainder banks forward.
            last=True runs this pair's ctx regions eagerly inside the j-loop
            and its normalize/transpose chain immediately after, shortening
            the post-exp tail."""
            nkt = 4 * stripe + 4
            attn_state[(stripe, pr)] = []
            parts = attn_ph2_parts(stripe, pr, last=True) if last else None
            # deferred prerequisite of this pair's PENDING pops (not of ph1):
            # run it lazily so the first ph1/exp is already emitted before
            # the forced PE burst
            pre = [pre_pending] if pre_pending else []
            for j in range(nkt):
                attn_ph1(stripe, pr, j)
                if last and j >= 4 * stripe:
                    # staggered eager ph2: the bulk of each region (whose e
                    # tiles are all >=1 round old, so the in-order PE never
                    # stalls on a fresh exp) runs 2 rounds early; only a
                    # 2-matmul tail + recip/normalize trail each exp
                    d = j - 4 * stripe
                    if d == 0:
                        # the previous pair's ph2 must be fully emitted
                        # before this pair claims the single ctx PSUM slot
                        while pending:
                            pending.popleft()[1]()
                        parts["zero"]()
                        for qb in (0, 1):
                            parts["early"](0, qb)
                            parts["early"](1, qb)
                    else:
                        parts["tail"](0, d - 1)
                        parts["tail"](1, d - 1)
                        parts["early_finish"](d - 1)
                        if d + 1 <= 3:
                            parts["early"](0, d + 1)
                            parts["early"](1, d + 1)
                if last and j == nkt - 1:
                    # nothing may delay the final normalize/close chain
                    continue
                budget = carry[0] + budget_ns
                while budget > 100:
                    if pending:
                        if pre:
                            ensure(pre.pop())
                        ns, t = pending.popleft()
                        t()
                    elif fillers:
                        ns = pop_filler()
                    else:
                        break
                    budget -= max(ns, 50)
                carry[0] = min(max(budget, 0), B_CARRY)
            if last:
                parts["tail"](0, 3)
                parts["tail"](1, 3)
                parts["early_finish"](3)
                for _, t in parts["units"]:
                    t()
            else:
                pending.extend(attn_ph2_units(stripe, pr))

        def oproj_tiles(t8, alt=False, quarters=False):
            """Thunks for one 128-token output block: per 512-dout half,
            6 DR instrs (2 main chunk-pairs + 4 cross), ~640ns, or three
            ~213ns thirds (quarters=True)."""
            osl = slice(t8 * P, (t8 + 1) * P)
            state = {}

            def mm_main(ps, cs, start):
                for c in cs:
                    nc.tensor.matmul(ps[:], lhsT=cd_s[:, c:c + 2, 0, osl],
                                     rhs=wo_s[:, c:c + 2, 1,
                                              state["dsl"]],
                                     start=(start and c == cs[0]), stop=False,
                                     perf_mode=DR)

            def mm_cross(ps, kcs, stop):
                for kc in kcs:
                    nc.tensor.matmul(ps[:], lhsT=cd_s[:, kc, 0:2, osl],
                                     rhs=wo_s[:, kc, 0:2, state["dsl"]],
                                     start=False, stop=(stop and kc == kcs[-1]),
                                     perf_mode=DR)

            def fin(dt, ps):
                # both 512-halves stage into one tile; a single [128, 1024]
                # DMA per token block halves HWDGE/semaphore traffic
                if dt == 0:
                    state["ob"] = spool.tile([P, 2, QW], BF16, tag="outsb",
                                             name="ob", bufs=3)
                ob = state["ob"]
                nc.vector.tensor_copy(out=ob[:, dt, :], in_=ps[:])
                if dt == 1:
                    (nc.scalar if alt else nc.sync).dma_start(
                        out[osl, :], ob[:])
                    del state["ob"]

            def whole(dt):
                state["dsl"] = slice(dt * QW, (dt + 1) * QW)
                if alt and dt == 1:
                    ps = pp_sc.tile([P, 2 * QW], F32, tag="sc",
                                    name="oproj_ps")[:, 0:QW]
                else:
                    ps = pp_acc.tile([P, QW], F32, tag="acc", name="oproj_ps")
                mm_main(ps, [0, 2], True)
                mm_cross(ps, range(KC2), True)
                fin(dt, ps)

            def qopen(dt):
                state["dsl"] = slice(dt * QW, (dt + 1) * QW)
                state[dt] = pp_acc.tile([P, QW], F32, tag="acc",
                                        name="oproj_ps")
                mm_main(state[dt], [0, 2], True)
                mm_cross(state[dt], [0, 1], False)

            def qclose(dt):
                state["dsl"] = slice(dt * QW, (dt + 1) * QW)
                ps = state.pop(dt)
                mm_cross(ps, [2, 3], True)
                fin(dt, ps)

            if quarters:
                return [lambda dt=dt, f=f: f(dt)
                        for dt in range(2) for f in (qopen, qclose)]
            return [lambda dt=dt: whole(dt) for dt in range(2)]

        def oproj(t8, alt=False):
            for t in oproj_tiles(t8, alt):
                t()

        heldpart = {}

        def oproj_openA(t8, dt):
            """Pairs 0/1 products of a final oproj tile -- legal as soon as
            cdig(3,1) has popped, i.e. inside pair (3,2)'s window where the
            PE otherwise starves. Staged to SBUF bf16; the close replays
            pair 2 + the bf16 last-pair products and adds this partial."""
            ps = pp_acc.tile([P, QW], F32, tag="acc", name="oproj_ps")
            dsl = slice(dt * QW, (dt + 1) * QW)
            osl = slice(t8 * P, (t8 + 1) * P)
            nc.tensor.matmul(ps[:], lhsT=cd_s[:, 0:2, 0, osl],
                             rhs=wo_s[:, 0:2, 1, dsl],
                             start=True, stop=False, perf_mode=DR)
            for kc in range(2):
                nc.tensor.matmul(ps[:], lhsT=cd_s[:, kc, 0:2, osl],
                                 rhs=wo_s[:, kc, 0:2, dsl],
                                 start=False, stop=(kc == 1), perf_mode=DR)
            part = spool.tile([P, QW], BF16, tag="opart",
                              name=f"opart{t8}_{dt}", bufs=8)
            nc.vector.tensor_copy(out=part[:], in_=ps[:])
            heldpart[(t8, dt)] = part

        heldob = {}

        def oproj_close(t8, dt):
            part = heldpart.pop((t8, dt))
            cbf = lastcbf[0]
            tloc = slice((t8 - 12) * P, (t8 - 11) * P)
            ps = pp_acc.tile([P, QW], F32, tag="acc", name="oproj_ps")
            dsl = slice(dt * QW, (dt + 1) * QW)
            osl = slice(t8 * P, (t8 + 1) * P)
            nc.tensor.matmul(ps[:], lhsT=cd_s[:, 2, 0:2, osl],
                             rhs=wo_s[:, 2, 0:2, dsl],
                             start=True, stop=False, perf_mode=DR)
            nc.tensor.matmul(ps[:], lhsT=cd_s[:, 2, 0, osl],
                             rhs=wo_s[:, 2, 1, dsl],
                             start=False, stop=False)
            nc.tensor.matmul(ps[:], lhsT=cbf[:, tloc],
                             rhs=wo_s[:, 3, 1, dsl],
                             start=False, stop=False)
            nc.tensor.matmul(ps[:], lhsT=cbf[:, tloc],
                             rhs=wo_s[:, 3, 0, dsl],
                             start=False, stop=True)
            if dt == 0:
                heldob[t8] = spool.tile([P, 2, QW], BF16, tag="outsb",
                                        name="ob", bufs=3)
            ob = heldob[t8]
            nc.vector.tensor_tensor(ob[:, dt, :], ps[:], part[:],
                                    mybir.AluOpType.add)
            # per-half DMA: each half ships as soon as its add lands
            (nc.sync if t8 % 2 else nc.scalar).dma_start(out[osl, dsl],
                                                         ob[:, dt, :])
            if dt == 1:
                del heldob[t8]

        # ---- schedule ----
        # Startup DMAs: the minimal set for pair (0,0) first (xt0, m0 of
        # Wq/Wk, biases, pad, tri), then attention starts while the rest of
        # the weights stream in and the remaining stage work rides fillers.
        xt0 = xpool.tile([P, KC, 2, QW], F8, tag="xt")
        nc.scalar.dma_start(xt0[:, 0:2, :, :], xt[:, 0, 0:2, :, :])
        nc.sync.dma_start(xt0[:, 4:6, :, :], xt[:, 0, 4:6, :, :])
        # PE warmup on a zeroed tile: keeps the PE continuously busy through
        # the startup DMA shadow so the first real matmuls run at full clock
        # (the cost model's p-state ramp needs ~3us of uninterrupted work)
        wmm = wpool.tile([P, QW], BF16, tag="wmm")
        nc.gpsimd.memset(wmm[:], 0.0)
        wps = pp_sc.tile([P, 2 * QW], F32, tag="sc", name="warm_ps")
        for i in range(WARM_N):
            nc.tensor.matmul(wps[:, 0:QW], lhsT=wmm[:, 0:P], rhs=wmm[:],
                             start=(i == 0), stop=(i == WARM_N - 1))
        nc.scalar.dma_start(xt0[:, 2:4, :, :], xt[:, 0, 2:4, :, :])
        nc.sync.dma_start(wk_s[:, 0, :, :, :], wk[:, 0, :, :, :])
        nc.scalar.dma_start(xt0[:, 6:8, :, :], xt[:, 0, 6:8, :, :])
        nc.sync.dma_start(wq_s[:, 0, :, :, :], wq[:, 0, :, :, :])
        nc.scalar.dma_start(bk_s[:], bkp[:])
        nc.sync.dma_start(bq_s[:], bqp[:])
        nc.scalar.dma_start(tri_s[:], tri[:])
        nc.sync.dma_start(pad_s[:], pad[:])
        # q digits on DVE, k digits on Pool: the two first-pair digit chains
        # run on different engines in parallel
        qkv_stage(0, xt0, parts="q", ms=[0], dig_dve=True)
        qkv_stage(0, xt0, parts="k", ms=[0])
        nc.sync.dma_start(wv_s[:, 0:4, :, :], wv[:, 0:4, :, :])
        nc.scalar.dma_start(wv_s[:, 4:8, :, :], wv[:, 4:8, :, :])
        xt1 = load_xt(1)
        nc.sync.dma_start(wq_s[:, 1, :, :, :], wq[:, 1, :, :, :])
        nc.scalar.dma_start(wk_s[:, 1, :, :, :], wk[:, 1, :, :, :])
        nc.sync.dma_start(wq_s[:, 2, :, :, :], wq[:, 2, :, :, :])
        nc.scalar.dma_start(wk_s[:, 2, :, :, :], wk[:, 2, :, :, :])
        nc.sync.dma_start(wq_s[:, 3, :, :, :], wq[:, 3, :, :, :])
        nc.scalar.dma_start(wk_s[:, 3, :, :, :], wk[:, 3, :, :, :])
        nc.scalar.dma_start(eye_s[:], eye[:])

        # stripe 0/1 pair interleave: stripe 0 alone supplies too little exp
        # work to keep ACT busy through the projection-heavy opening, so
        # stripe-1 pairs (2x the exp volume) run in between
        fillers.extend(qkv_tiles(1, xt1, parts="q", ms=[0]))
        fillers.extend(qkv_tiles(1, xt1, parts="k", ms=[0]))
        fillers.extend(qkv_tiles(0, xt0, parts="v"))
        fillers.extend(qkv_tiles(0, xt0, parts="q", ms=[1]))
        fillers.extend(qkv_tiles(0, xt0, parts="k", ms=[1]))
        fillers.extend(qkv_tiles(1, xt1, parts="v"))
        for m in (1, 2, 3):
            sts = (1,) if m == 1 else (0, 1)
            for stq in sts:
                xtt = xt0 if stq == 0 else xt1
                fillers.extend(qkv_tiles(stq, xtt, parts="q", ms=[m]))
                fillers.extend(qkv_tiles(stq, xtt, parts="k", ms=[m]))
        nc.sync.dma_start(wo_s[:], wo[:])
        attn_pair(0, 0, budget_ns=B_P00)
        ensure((1, "q", 0))
        ensure((1, "k", 0))
        ensure((0, "v", 3))
        attn_pair(1, 0, budget_ns=B_EARLY)
        ensure((0, "q", 1))
        ensure((0, "k", 1))
        ensure((1, "v", 3))
        attn_pair(0, 1, budget_ns=B_EARLY)
        ensure((1, "q", 1))
        ensure((1, "k", 1))
        attn_pair(1, 1, budget_ns=B_EARLY)
        ensure((0, "q", 2))
        ensure((0, "k", 2))
        attn_pair(0, 2, budget_ns=B_EARLY)
        ensure((1, "q", 2))
        ensure((1, "k", 2))
        attn_pair(1, 2, budget_ns=B_EARLY)
        ensure((0, "q", 3))
        ensure((0, "k", 3))
        attn_pair(0, 3, budget_ns=B_EARLY)
        ensure((1, "q", 3))
        ensure((1, "k", 3))
        xt2 = load_xt(2)

        def extend_stage_qk(st, xt_t):
            for m in range(KC2):
                fillers.extend(qkv_tiles(st, xt_t, parts="q", ms=[m]))
                fillers.extend(qkv_tiles(st, xt_t, parts="k", ms=[m]))

        extend_stage_qk(2, xt2)
        fillers.extend(qkv_tiles(2, xt2, parts="v"))
        attn_pair(1, 3, budget_ns=B_EARLY)
        xt3 = load_xt(3)
        extend_stage_qk(3, xt3)
        for pr in range(4):
            ensure((2, "q", pr))
            ensure((2, "k", pr))
            if pr == 1:
                ensure((2, "v", 3))
            attn_pair(2, pr, budget_ns=B_MID)
        # V(3) is safe here -- first needed by the ctx regions of pair (3,0),
        # which only run during ph1(3,1). oproj of stripe-2 blocks becomes
        # legal once pair (2,3)'s pending units pop at the start of (3,0).
        fillers.extend(qkv_tiles(3, xt3, parts="v"))
        for t8 in range(0, 11):
            fillers.extend((213, t, None)
                           for t in oproj_tiles(t8, quarters=True))
        for pr in range(2):
            ensure((3, "q", pr))
            ensure((3, "k", pr))
            if pr == 1:
                ensure((3, "v", 3))
            attn_pair(3, pr, budget_ns=B_S3)
        # openA (pairs 0/1 products) of the final oproj tiles becomes legal
        # once cdig(3,1) pops -- feed it to pair (3,2)'s otherwise-starved
        # PE, together with the remaining stripe-2 oproj tile
        ensure((3, "q", 2))
        ensure((3, "k", 2))
        fillers.extend((213, t, None)
                       for t in oproj_tiles(11, quarters=True))
        for t8 in range(12, 16):
            fillers.append((320, (lambda t8=t8: oproj_openA(t8, 0)), None))
            fillers.append((320, (lambda t8=t8: oproj_openA(t8, 1)), None))
        attn_pair(3, 2, budget_ns=B_S3)
        ensure((3, "q", 3))
        ensure((3, "k", 3))
        attn_pair(3, 3, budget_ns=B_LAST, last=True)
        while pending:
            pending.popleft()[1]()
        drain()
        for t8 in range(12, 16):
            oproj_close(t8, 0)
            oproj_close(t8, 1)

    nc.compile()
    return nc


def _dig(a):
    """two-digit e4m3 split along a new axis: returns np [..., 2] fp8"""
    hi = a.astype(NPE4)
    lo = (a - hi.astype(np.float32)).astype(NPE4)
    return hi, lo


def _core_inputs(c, x, padding_mask, Wq, bq, Wk, bk, Wv, bv, Wo, bo):
    b, hh = c // 2, c % 2
    hsl = slice(hh * 512, (hh + 1) * 512)

    xb = np.ascontiguousarray(
        x[b].T.reshape(KC, P, S).transpose(1, 0, 2)).astype(np.float32)
    x8, xr = _dig(xb)
    # [P, KC, 2, S] -> stripe-major [P, NS, KC, 2, QW]
    xt = np.stack([x8, xr], axis=2).reshape(P, KC, 2, NS, QW)
    xt = np.ascontiguousarray(xt.transpose(0, 3, 1, 2, 4))

    def wl(Wh):  # [512 out, 1024 in] -> m-major [P, KC2, KC, 2, 128] {Wr,W8}
        w = np.ascontiguousarray(
            Wh.T.reshape(KC, P, 512).transpose(1, 0, 2)).astype(np.float32)
        w8, wr = _dig(WS * w)
        st = np.stack([wr, w8], axis=2)          # [P, KC, 2, 512]
        st = st.reshape(P, KC, 2, KC2, P).transpose(0, 3, 1, 2, 4)
        return np.ascontiguousarray(st)

    def wvl(Wh):  # [512 out, 1024 in] -> chunk-major [P, KC, 2, 512] {Wr,W8}
        w = np.ascontiguousarray(
            Wh.T.reshape(KC, P, 512).transpose(1, 0, 2)).astype(np.float32)
        w8, wr = _dig(WS * w)
        return np.ascontiguousarray(np.stack([wr, w8], axis=2))

    wob = np.ascontiguousarray(
        Wo[:, hsl].T.reshape(KC2, P, D).transpose(1, 0, 2)).astype(np.float32)
    wo8, wor = _dig(WS * wob)
    wol = np.ascontiguousarray(np.stack([wor, wo8], axis=2))

    bqp = np.ascontiguousarray(
        WS * bq[hsl].reshape(KC2, P).T).astype(np.float32)
    bkp = np.ascontiguousarray(
        WS * bk[hsl].reshape(KC2, P).T).astype(np.float32)

    padb = np.where(padding_mask[b].reshape(S // P, P).T, 0.0,
                    NEG).astype(np.float32)
    padb = np.ascontiguousarray(padb)

    kk = np.arange(P)[:, None]
    uu = np.arange(QW)[None, :]
    trib = np.ascontiguousarray((kk <= uu).astype(NPBF16))

    return {"xt": xt, "wq": wl(Wq[hsl]), "wk": wl(Wk[hsl]), "wv": wvl(Wv[hsl]),
            "wo": wol, "bqp": bqp, "bkp": bkp, "pad": padb, "tri": trib,
            "eye": np.eye(P, dtype=NPBF16)}


_NC_CACHE = {}


def kernel(x, padding_mask, Wq, bq, Wk, bk, Wv, bv, Wo, bo):
    x = np.asarray(x, np.float32)
    padding_mask = np.asarray(padding_mask, bool)
    args = [np.asarray(a, np.float32) for a in (Wq, bq, Wk, bk, Wv, bv, Wo, bo)]

    if "nc" not in _NC_CACHE:
        _NC_CACHE["nc"] = _build()
    nc = _NC_CACHE["nc"]

    in_maps = [_core_inputs(c, x, padding_mask, *args) for c in range(8)]

    trace = bool(int(os.environ.get("KERNEL_TRACE", "0")))
    try:
        res = run_bass_kernel_spmd(nc, in_maps, core_ids=list(range(8)), trace=trace)
    except ModuleNotFoundError:
        res = run_bass_kernel_spmd(nc, in_maps, core_ids=list(range(8)))
    if trace and res.exec_time_ns is not None:
        print(f"HW exec time: {res.exec_time_ns} ns")
        _NC_CACHE["exec_time_ns"] = res.exec_time_ns

    Wo_, bv_, bo_ = args[6], args[5], args[7]
    btot = (bo_ + Wo_ @ bv_).astype(np.float32)
    descale = 1.0 / (WS * WS)
    full = np.empty((B, S, D), np.float32)
    for b in range(B):
        full[b] = ((res.results[2 * b]["out"].astype(np.float32)
                    + res.results[2 * b + 1]["out"].astype(np.float32))
                   * descale + btot)
    return full


if __name__ == "__main__":
    rng = np.random.default_rng(0)
    x = rng.standard_normal((B, S, D), dtype=np.float32)
    lengths = rng.integers(S // 2, S + 1, size=(B,))
    pm = np.arange(S)[None, :] < lengths[:, None]
    std = 0.02
    ws = {n: (rng.standard_normal((D, D), dtype=np.float32) * std)
          for n in ("Wq", "Wk", "Wv", "Wo")}
    z = np.zeros((D,), np.float32)
    out = kernel(x, pm, ws["Wq"], z, ws["Wk"], z, ws["Wv"], z, ws["Wo"], z)
    print(out.shape, out.dtype, np.abs(out).mean())
